# revision 23
# baseline (speedup 1.0000x reference)
"""Trainium2 Bass kernel for nn_Attention_40767829574453 (Glow-TTS style aligner).

Sharding: pure data parallelism, batch 16 -> 8 cores x 2 batches each.

Device (per core): both 6-layer conv stacks (masked conv1d as shifted matmuls +
groupnorm folded into a Relu activation + speaker/gst layer bias + residuals),
key/query projections + L2 normalization, similarity bmm, CTC softmax /
log-softmax, sim_ctc bmm (one-hot gather as matmul), softplus log-sigmoid
terms, masked reductions, BOTH monotonic-alignment forward DPs (rows of the DP
computed with hardware tensor_tensor_scan in a negated min/add form) and the
backtrack decision bits D = (i==j) | (v[i-1,j-1] > v[i,j-1]).

Host: tiny exact backtrack walks over the D bits, log-space CTC scan, scalar
loss assembly, one-hot attention build.
"""
import numpy as np

import concourse.bass as bass
import concourse.bacc as bacc
import concourse.mybir as mybir
import concourse.tile as tile
from concourse.bass_utils import run_bass_kernel_spmd
from concourse.masks import make_identity

F32 = mybir.dt.float32
U8 = mybir.dt.uint8
I32 = mybir.dt.int32
AF = mybir.ActivationFunctionType
OP = mybir.AluOpType
AX = mybir.AxisListType

N, TX, TDEC = 16, 256, 1024
MEL = 80
ATT_H = 128
VOCAB = 100
NL = 6
C = 256
NCORES = 8
NB = 2
BIGPOS = 1e9
TP = TDEC + 2
VB = TDEC + 1  # vtab block width (guard col + 1024)


# ----------------------------------------------------------------------------
# device program
# ----------------------------------------------------------------------------

def build_nc(phase=None):
    import os
    phase = phase or os.environ.get("KPHASE", "all")
    P = ["conv", "sim", "ctc", "scan", "d", "all"].index(phase)
    nc = bacc.Bacc("TRN2")
    dt = nc.dram_tensor
    ins = dict(
        xm=dt("xm", [NB, MEL, TP], F32, kind="ExternalInput"),
        spec_y=dt("spec_y", [NB, 8, 128, MEL], F32, kind="ExternalInput"),
        textT=dt("textT", [NB, 257, TX], F32, kind="ExternalInput"),
        eWt=dt("eWt", [257, ATT_H], F32, kind="ExternalInput"),
        qWt=dt("qWt", [257, ATT_H - 1], F32, kind="ExternalInput"),
        cWt=dt("cWt", [257, VOCAB], F32, kind="ExternalInput"),
        w0t_m=dt("w0t_m", [3, MEL, C], F32, kind="ExternalInput"),
        w0t_a=dt("w0t_a", [3, MEL, C], F32, kind="ExternalInput"),
        wrt_m=dt("wrt_m", [NL - 1, 3, C, C], F32, kind="ExternalInput"),
        wrt_a=dt("wrt_a", [NL - 1, 3, C, C], F32, kind="ExternalInput"),
        gnw_m=dt("gnw_m", [NL, C], F32, kind="ExternalInput"),
        gnb_m=dt("gnb_m", [NL, C], F32, kind="ExternalInput"),
        gnw_a=dt("gnw_a", [NL, C], F32, kind="ExternalInput"),
        gnb_a=dt("gnb_a", [NL, C], F32, kind="ExternalInput"),
        lb_m=dt("lb_m", [NL, NB, C], F32, kind="ExternalInput"),
        lb_a=dt("lb_a", [NL, NB, C], F32, kind="ExternalInput"),
        smask=dt("smask", [NB, TP], F32, kind="ExternalInput"),
        tmask=dt("tmask", [NB, TX], F32, kind="ExternalInput"),
        ehotT=dt("ehotT", [NB, VOCAB, TX], F32, kind="ExternalInput"),
        consts=dt("consts", [1, 8], F32, kind="ExternalInput"),
        rowidx=dt("rowidx", [128, 8], F32, kind="ExternalInput"),
    )
    outs = dict(
        o_sim=dt("o_sim", [NB, 2, 128, TDEC], F32, kind="ExternalOutput"),
        o_lsm=dt("o_lsm", [NB, 8, 128, VOCAB], F32, kind="ExternalOutput"),
        o_db=dt("o_db", [128, 8 * TDEC], U8, kind="ExternalOutput"),
        o_mask=dt("o_mask", [NB, 2, 128, TDEC], F32, kind="ExternalOutput"),
        o_snls=dt("o_snls", [NB, 1], F32, kind="ExternalOutput"),
    )

    with tile.TileContext(nc) as tc:
        with (
            tc.tile_pool(name="per", bufs=1) as per,      # persistent, unique tags
            tc.tile_pool(name="xp", bufs=5) as xp,        # conv activations
            tc.tile_pool(name="wt", bufs=9) as wtp,      # conv weights
            tc.tile_pool(name="st", bufs=6) as stp,       # [128,1]-ish stats
            tc.tile_pool(name="rot", bufs=3) as rot,      # rotating [128,1024] work
            tc.tile_pool(name="cps", bufs=4, space="PSUM") as psp,   # [128,512]
            tc.tile_pool(name="sps", bufs=3, space="PSUM") as psp2,  # small psums
            tc.tile_pool(name="dps", bufs=1, space="PSUM") as psp3,  # PE observer scratch
        ):
            ident = per.tile([128, 128], F32, tag="ident")
            make_identity(nc, ident)
            pescr = psp3.tile([1, 1], F32, tag="pescr")

            def pe_obs(ap):
                # tiny matmul so the PE observes this tile's producer semaphore
                # before real matmuls (ISA wait-slot budget is ~1 per matmul).
                nc.tensor.matmul(pescr, ap[0:1, 0:1], ap[0:1, 0:1], start=True, stop=True)
            ones_col = per.tile([128, 1], F32, tag="ones_col")
            nc.vector.memset(ones_col, 1.0)
            ones_row = per.tile([1, 128], F32, tag="ones_row")
            nc.vector.memset(ones_row, 1.0)
            bigpos = per.tile([128, TDEC], F32, tag="bigpos")
            nc.vector.memset(bigpos, BIGPOS)
            jio = per.tile([128, TDEC], F32, tag="jio")
            nc.gpsimd.iota(jio, pattern=[[1, TDEC]], base=0, channel_multiplier=0, allow_small_or_imprecise_dtypes=True)
            csts = per.tile([128, 8], F32, tag="csts")
            nc.gpsimd.dma_start(out=csts, in_=bass.AP(ins["consts"], 0, [[0, 128], [1, 8]]))
            sc10 = csts[:, 0:1]
            smb = csts[:, 1:2]
            epsc = csts[:, 2:3]
            rix = per.tile([128, 8], F32, tag="rix")
            nc.sync.dma_start(out=rix, in_=ins["rowidx"][:, :])

            gw_m = per.tile([128, NL * 2], F32, tag="gw_m")
            gb_m = per.tile([128, NL * 2], F32, tag="gb_m")
            gw_a = per.tile([128, NL * 2], F32, tag="gw_a")
            gb_a = per.tile([128, NL * 2], F32, tag="gb_a")
            lbm = per.tile([128, NL * NB * 2], F32, tag="lbm")
            lba = per.tile([128, NL * NB * 2], F32, tag="lba")
            # single strided DMA each: [128 part, (l, ct)] / [128, (l, b, ct)]
            for t_sb, t_dram in ((gw_m, "gnw_m"), (gb_m, "gnb_m"), (gw_a, "gnw_a"), (gb_a, "gnb_a")):
                nc.sync.dma_start(out=t_sb[:, :],
                                  in_=bass.AP(ins[t_dram], 0, [[1, 128], [C, NL], [128, 2]]))
            for t_sb, t_dram in ((lbm, "lb_m"), (lba, "lb_a")):
                nc.sync.dma_start(out=t_sb[:, :],
                                  in_=bass.AP(ins[t_dram], 0, [[1, 128], [NB * C, NL], [C, NB], [128, 2]]))

            maskbc = []
            for b in range(NB):
                mb = per.tile([128, TDEC], F32, tag=f"maskbc{b}")
                nc.gpsimd.dma_start(
                    out=mb, in_=bass.AP(ins["smask"], b * TP + 1, [[0, 128], [1, TDEC]]))
                maskbc.append(mb)
            tmcol = per.tile([128, NB * 2], F32, tag="tmcol")
            nc.sync.dma_start(out=tmcol[:, :],
                              in_=bass.AP(ins["tmask"], 0, [[1, 128], [TX, NB], [128, 2]]))
            # absorb the small-param DMA semaphores into DVE's clock so later
            # tensor_scalar ops (1 wait slot in ISA) need no DMA waits
            obs = per.tile([128, 8], F32, tag="obs")
            for i_, t_ in enumerate((gw_m, gb_m, gw_a, gb_a, lbm, lba, tmcol, rix)):
                nc.vector.tensor_copy(out=obs[:, i_:i_ + 1], in_=t_[:, 0:1])

            d_val = nc.dram_tensor("d_val", [4, TX, TDEC], F32)
            d_vt = nc.dram_tensor("d_vt", [4, TX, VB], F32)
            NSLOT = 6
            slotring = per.tile([128, NSLOT * VB], F32, tag="slotring")
            valring = per.tile([128, NSLOT * TDEC], F32, tag="valring")
            simall = per.tile([128, 4 * TDEC], F32, tag="simall")
            m2all = per.tile([128, 4 * TDEC], F32, tag="m2all")
            lsmall = per.tile([128, 16 * VOCAB], F32, tag="lsmall")
            snall = per.tile([1, NB], F32, tag="snall")
            for t_ in (simall, m2all, lsmall, snall, valring, slotring):
                nc.vector.memset(t_, 0.0)


            def conv_stack(b, aux):
                w0 = ins["w0t_a" if aux else "w0t_m"]
                wr = ins["wrt_a" if aux else "wrt_m"]
                gw = gw_a if aux else gw_m
                gb = gb_a if aux else gb_m
                lb = lba if aux else lbm
                x0 = xp.tile([128, TP], F32, tag="x")
                nc.sync.dma_start(out=x0[:MEL, :], in_=ins["xm"][b])
                pe_obs(x0)
                x = [x0]
                for l in range(NL):
                    cin_tiles = 1 if l == 0 else 2
                    kdim = MEL if l == 0 else 128
                    wts = []
                    for k in range(3):
                        for cit in range(cin_tiles):
                            w = wtp.tile([128, C], F32, tag="wt")
                            if l == 0:
                                nc.sync.dma_start(out=w[:MEL, :], in_=w0[k])
                            else:
                                nc.sync.dma_start(out=w, in_=wr[l - 1, k, cit * 128:(cit + 1) * 128, :])
                            wts.append(w)
                    for w in wts:
                        pe_obs(w)
                    for xt__ in x:
                        pe_obs(xt__)
                    xn = []
                    for ct in range(2):
                        stats = stp.tile([128, 2, 6], F32, tag="bnst")
                        pss = []
                        for lc in range(2):
                            ps = psp.tile([128, 512], F32, tag="cps")
                            mm = 0
                            for k in range(3):
                                for cit in range(cin_tiles):
                                    nc.tensor.matmul(
                                        ps,
                                        wts[k * cin_tiles + cit][:kdim, ct * 128:(ct + 1) * 128],
                                        x[cit][:kdim, lc * 512 + k: lc * 512 + k + 512],
                                        start=(mm == 0), stop=(mm == 3 * cin_tiles - 1))
                                    mm += 1
                            nc.vector.bn_stats(out=stats[:, lc, :], in_=ps[:, :])
                            pss.append(ps)
                        mv = stp.tile([128, 2], F32, tag="bnmv")
                        nc.vector.bn_aggr(out=mv, in_=stats)
                        sd = stp.tile([128, 1], F32, tag="sd")
                        nc.scalar.activation(out=sd, in_=mv[:, 1:2], func=AF.Sqrt, bias=epsc)
                        rs = stp.tile([128, 1], F32, tag="rs")
                        nc.vector.reciprocal(rs, sd)
                        ga = stp.tile([128, 1], F32, tag="ga")
                        nc.vector.tensor_mul(ga, rs, gw[:, l * 2 + ct: l * 2 + ct + 1])
                        gbb = stp.tile([128, 1], F32, tag="gbb")
                        nc.vector.tensor_mul(gbb, mv[:, 0:1], ga)
                        nc.vector.tensor_sub(gbb, gb[:, l * 2 + ct: l * 2 + ct + 1], gbb)
                        xt_ = xp.tile([128, TP], F32, tag="x")
                        for lc in range(2):
                            nc.scalar.activation(out=xt_[:, 1 + lc * 512: 1 + (lc + 1) * 512],
                                                 in_=pss[lc], func=AF.Relu, scale=ga, bias=gbb)
                        lcol = lb[:, (l * NB + b) * 2 + ct: (l * NB + b) * 2 + ct + 1]
                        nc.vector.tensor_scalar_add(xt_[:, 1:1 + TDEC], xt_[:, 1:1 + TDEC], lcol)
                        if l % 2 == 1:
                            nc.vector.tensor_add(xt_[:, 1:1 + TDEC], xt_[:, 1:1 + TDEC],
                                                 x[ct][:, 1:1 + TDEC])
                        if l < NL - 1:
                            nc.vector.tensor_mul(xt_[:, 1:1 + TDEC], xt_[:, 1:1 + TDEC], maskbc[b])
                        nc.vector.memset(xt_[:, 0:1], 0.0)
                        nc.vector.memset(xt_[:, TP - 1: TP], 0.0)
                        xn.append(xt_)
                    x = xn
                return x

            def normalize_rows(t, n_free):
                scr = rot.tile([128, n_free], F32, tag="sw")
                sq = stp.tile([128, 1], F32, tag="nsq")
                nc.vector.tensor_mul(scr, t[:, :n_free], t[:, :n_free])
                nc.vector.reduce_sum(sq, scr, axis=AX.X)
                sd = stp.tile([128, 1], F32, tag="nsd")
                nc.scalar.activation(out=sd, in_=sq, func=AF.Sqrt)
                rr = stp.tile([128, 1], F32, tag="nrr")
                nc.vector.reciprocal(rr, sd)
                nc.vector.tensor_scalar_mul(t[:, :n_free], t[:, :n_free], rr)

            def stage_val(val, b, m, xt):
                p = 2 * m + b
                nc.sync.dma_start(out=d_val[p, xt * 128:(xt + 1) * 128, :], in_=val)

            # ---------------- main stacks, key/query, similarity ----------------
            m2_tiles = {}
            for b in range(NB):
                x6 = conv_stack(b, aux=False)
                if P < 1:
                    continue
                SUB = int(os.environ.get("KSUB", "9"))

                # key
                tta = [per.tile([128, TX], F32, tag=f"tt{i}", name=f"tt{i}") for i in range(2)]
                ttb = per.tile([1, TX], F32, tag="ttb")
                for i in range(2):
                    nc.sync.dma_start(out=tta[i], in_=ins["textT"][b, i * 128:(i + 1) * 128, :])
                nc.sync.dma_start(out=ttb, in_=ins["textT"][b, 256:257, :])
                pe_obs(tta[0]); pe_obs(tta[1]); pe_obs(ttb)
                if b == 0:
                    eW = [per.tile([128, ATT_H], F32, tag=f"eW{i}", name=f"eW{i}") for i in range(2)]
                    eWb = per.tile([1, ATT_H], F32, tag="eWb")
                    for i in range(2):
                        nc.sync.dma_start(out=eW[i], in_=ins["eWt"][i * 128:(i + 1) * 128, :])
                    nc.sync.dma_start(out=eWb, in_=ins["eWt"][256:257, :])
                    qW = [per.tile([128, ATT_H - 1], F32, tag=f"qW{i}", name=f"qW{i}") for i in range(2)]
                    qWb = per.tile([1, ATT_H - 1], F32, tag="qWb")
                    for i in range(2):
                        nc.sync.dma_start(out=qW[i], in_=ins["qWt"][i * 128:(i + 1) * 128, :])
                    nc.sync.dma_start(out=qWb, in_=ins["qWt"][256:257, :])
                    for t_ in (eW[0], eW[1], eWb, qW[0], qW[1], qWb):
                        pe_obs(t_)
                keyT = per.tile([128, TX], F32, tag=f"keyT{b}")
                nc.vector.memset(keyT, 0.0)
                for xt in range(2):
                    if SUB < 1:
                        continue
                    pk = psp2.tile([128, ATT_H], F32, tag="sp")
                    for i in range(2):
                        nc.tensor.matmul(pk, tta[i][:, xt * 128:(xt + 1) * 128], eW[i],
                                         start=(i == 0), stop=False)
                    nc.tensor.matmul(pk, ttb[:, xt * 128:(xt + 1) * 128], eWb,
                                     start=False, stop=True)
                    kk = rot.tile([128, ATT_H], F32, tag="sw")
                    nc.scalar.activation(out=kk, in_=pk, func=AF.Copy)
                    if SUB >= 2:
                        normalize_rows(kk, ATT_H)
                    if SUB >= 3:
                        pt = psp2.tile([128, 128], F32, tag="sp")
                        nc.tensor.transpose(pt, kk, ident)
                        nc.scalar.activation(out=keyT[:, xt * 128:(xt + 1) * 128], in_=pt, func=AF.Copy)

                if SUB < 4:
                    continue
                # query
                qT = per.tile([128, TDEC], F32, tag="qT")
                for yt in range(8):
                    pq = psp2.tile([128, ATT_H - 1], F32, tag="sp")
                    for i in range(2):
                        nc.tensor.matmul(pq, x6[i][:, 1 + yt * 128: 1 + (yt + 1) * 128], qW[i],
                                         start=(i == 0), stop=False)
                    nc.tensor.matmul(pq, ones_row, qWb,
                                     start=False, stop=True)
                    qq = rot.tile([128, 128], F32, tag="sw")
                    nc.scalar.activation(out=qq[:, 0:ATT_H - 1], in_=pq, func=AF.Copy)
                    sy = rot.tile([128, MEL], F32, tag="sw")
                    nc.sync.dma_start(out=sy, in_=ins["spec_y"][b, yt])
                    en = stp.tile([128, 1], F32, tag="en")
                    nc.vector.reduce_sum(en, sy, axis=AX.X)
                    nc.vector.tensor_scalar_mul(qq[:, ATT_H - 1: ATT_H], en, 1.0 / MEL)
                    normalize_rows(qq, ATT_H)
                    pt = psp2.tile([128, 128], F32, tag="sp")
                    nc.tensor.transpose(pt, qq, ident)
                    nc.scalar.activation(out=qT[:, yt * 128:(yt + 1) * 128], in_=pt, func=AF.Copy)

                if SUB < 5:
                    continue
                # similarity + masked softplus sums + val1
                snps = []
                for xt in range(2):
                    bx = b * 2 + xt
                    ssim = simall[:, bx * TDEC:(bx + 1) * TDEC]
                    for yc in range(2):
                        psim = psp.tile([128, 512], F32, tag="cps")
                        nc.tensor.matmul(psim, keyT[:, xt * 128:(xt + 1) * 128],
                                         qT[:, yc * 512:(yc + 1) * 512], start=True, stop=True)
                        nc.scalar.activation(out=ssim[:, yc * 512:(yc + 1) * 512], in_=psim,
                                             func=AF.Identity, scale=sc10, bias=smb)
                    m2 = m2all[:, bx * TDEC:(bx + 1) * TDEC]
                    nc.vector.tensor_scalar_mul(m2, maskbc[b], tmcol[:, b * 2 + xt: b * 2 + xt + 1])
                    m2_tiles[(b, xt)] = m2
                    # nlsn = softplus(sim) = relu(sim) + ln(1 + exp(-|sim|))
                    ab = rot.tile([128, TDEC], F32, tag="work")
                    nc.scalar.activation(out=ab, in_=ssim, func=AF.Abs)
                    nc.scalar.activation(out=ab, in_=ab, func=AF.Exp, scale=-1.0)
                    nc.scalar.activation(out=ab, in_=ab, func=AF.Ln, bias=ones_col)
                    nlsn = rot.tile([128, TDEC], F32, tag="work")
                    nc.scalar.activation(out=nlsn, in_=ssim, func=AF.Relu)
                    nc.vector.tensor_add(nlsn, nlsn, ab)
                    scr = rot.tile([128, TDEC], F32, tag="work")
                    snp = stp.tile([128, 1], F32, tag=f"snp{xt}")
                    nc.vector.tensor_mul(scr, nlsn, m2)
                    nc.vector.reduce_sum(snp, scr, axis=AX.X)
                    snps.append(snp)
                    # softplus(-sim) = softplus(sim) - sim
                    lsn = rot.tile([128, TDEC], F32, tag="work")
                    nc.vector.tensor_sub(lsn, nlsn, ssim)
                    val = rot.tile([128, TDEC], F32, tag="work")
                    nc.vector.tensor_mul(val, lsn, m2)
                    stage_val(val, b, 0, xt)
                sn2 = stp.tile([128, 1], F32, tag="sn2")
                nc.vector.tensor_add(sn2, snps[0], snps[1])
                psn = psp2.tile([1, 1], F32, tag="sp")
                nc.tensor.matmul(psn, sn2, ones_col, start=True, stop=True)
                nc.scalar.activation(out=snall[:, b:b + 1], in_=psn, func=AF.Copy)

            # ---------------- aux stacks, ctc, sim_ctc, val2 ----------------
            for b in range(NB if P >= 2 else 0):
                x6a = conv_stack(b, aux=True)
                if b == 0:
                    cW = [per.tile([128, VOCAB], F32, tag=f"cW{i}", name=f"cW{i}") for i in range(2)]
                    cWb = per.tile([1, VOCAB], F32, tag="cWb")
                    for i in range(2):
                        nc.sync.dma_start(out=cW[i], in_=ins["cWt"][i * 128:(i + 1) * 128, :])
                    nc.sync.dma_start(out=cWb, in_=ins["cWt"][256:257, :])
                    for t_ in (cW[0], cW[1], cWb):
                        pe_obs(t_)
                qTc = per.tile([128, TDEC], F32, tag="qTc")
                for yt in range(8):
                    pc = psp2.tile([128, VOCAB], F32, tag="sp")
                    for i in range(2):
                        nc.tensor.matmul(pc, x6a[i][:, 1 + yt * 128: 1 + (yt + 1) * 128], cW[i],
                                         start=(i == 0), stop=False)
                    nc.tensor.matmul(pc, ones_row, cWb,
                                     start=False, stop=True)
                    mx = stp.tile([128, 1], F32, tag="mx")
                    nc.vector.reduce_max(mx, pc, axis=AX.X)
                    nmx = stp.tile([128, 1], F32, tag="nmx")
                    nc.vector.tensor_scalar_mul(nmx, mx, -1.0)
                    exb = rot.tile([128, VOCAB], F32, tag="sw")
                    nc.scalar.activation(out=exb, in_=pc, func=AF.Exp, bias=nmx)
                    s = stp.tile([128, 1], F32, tag="s")
                    nc.vector.reduce_sum(s, exb, axis=AX.X)
                    lns = stp.tile([128, 1], F32, tag="lns")
                    nc.scalar.activation(out=lns, in_=s, func=AF.Ln)
                    r = stp.tile([128, 1], F32, tag="r")
                    nc.vector.reciprocal(r, s)
                    qsb = rot.tile([128, VOCAB], F32, tag="sw")
                    nc.vector.tensor_scalar_mul(qsb, exb, r)
                    nb_ = stp.tile([128, 1], F32, tag="nb_")
                    nc.vector.tensor_add(nb_, mx, lns)
                    nc.vector.tensor_scalar_mul(nb_, nb_, -1.0)
                    lidx = b * 8 + yt
                    nc.scalar.activation(out=lsmall[:, lidx * VOCAB:(lidx + 1) * VOCAB],
                                         in_=pc, func=AF.Identity, bias=nb_)
                    ptc = psp2.tile([128, 128], F32, tag="sp")
                    nc.tensor.transpose(ptc[:VOCAB, :], qsb, ident)
                    nc.scalar.activation(out=qTc[:VOCAB, yt * 128:(yt + 1) * 128],
                                         in_=ptc[:VOCAB, :], func=AF.Copy)
                eh = per.tile([128, TX], F32, tag="eh")
                nc.sync.dma_start(out=eh[:VOCAB, :], in_=ins["ehotT"][b])
                pe_obs(eh)
                for xt in range(2):
                    scs = rot.tile([128, TDEC], F32, tag="work")
                    for yc in range(2):
                        psc = psp.tile([128, 512], F32, tag="cps")
                        nc.tensor.matmul(psc, eh[:VOCAB, xt * 128:(xt + 1) * 128],
                                         qTc[:VOCAB, yc * 512:(yc + 1) * 512],
                                         start=True, stop=True)
                        nc.scalar.activation(out=scs[:, yc * 512:(yc + 1) * 512], in_=psc,
                                             func=AF.Copy)
                    # softplus(-simc) = relu(-simc) + ln(1 + exp(-|simc|))
                    ab2 = rot.tile([128, TDEC], F32, tag="work")
                    nc.scalar.activation(out=ab2, in_=scs, func=AF.Abs)
                    nc.scalar.activation(out=ab2, in_=ab2, func=AF.Exp, scale=-1.0)
                    nc.scalar.activation(out=ab2, in_=ab2, func=AF.Ln, bias=ones_col)
                    lsn2 = rot.tile([128, TDEC], F32, tag="work")
                    nc.scalar.activation(out=lsn2, in_=scs, func=AF.Relu, scale=-1.0)
                    nc.vector.tensor_add(lsn2, lsn2, ab2)
                    val2 = rot.tile([128, TDEC], F32, tag="work")
                    nc.vector.tensor_mul(val2, lsn2, m2_tiles[(b, xt)])
                    stage_val(val2, b, 1, xt)

            # ---------------- MAS forward scans ----------------
            scanobs = per.tile([128, 2], F32, tag="scanobs")
            nc.vector.memset(scanobs, 0.0)
            for i in range(TX if P >= 3 else 0):
                s = i % NSLOT
                nc.sync.dma_start(out=valring[0:4, s * TDEC:(s + 1) * TDEC],
                                  in_=d_val[0:4, i, :])
                # absorb the valring-DMA sem and the slot WAR (vs d_vt DMA-out)
                nc.vector.tensor_copy(out=scanobs[0:4, 0:1],
                                      in_=valring[0:4, s * TDEC: s * TDEC + 1])
                nc.vector.memset(slotring[0:4, s * VB: s * VB + 1], BIGPOS)
                if i == 0:
                    data0 = bigpos[0:4, :]
                    init = 0.0
                else:
                    sp_ = (i - 1) % NSLOT
                    data0 = slotring[0:4, sp_ * VB: sp_ * VB + TDEC]
                    init = BIGPOS
                nc.vector.tensor_tensor_scan(
                    out=slotring[0:4, s * VB + 1: s * VB + 1 + TDEC],
                    data0=data0,
                    data1=valring[0:4, s * TDEC:(s + 1) * TDEC],
                    initial=init, op0=OP.min, op1=OP.add)
                nc.sync.dma_start(out=d_vt[0:4, i, :], in_=slotring[0:4, s * VB: (s + 1) * VB])

            # ---------------- D bits ----------------
            dbu = per.tile([128, 8 * TDEC], U8, tag="dbu")
            nc.vector.memset(dbu, 0)
            for g in range(2 if P >= 4 else 0):
                band = rot.tile([128, TDEC], F32, tag="work", name=f"band{g}")
                nc.vector.tensor_scalar(band, jio, rix[:, g:g + 1], None, op0=OP.is_equal)
                for p in range(4):
                    Ab = rot.tile([128, VB], F32, tag="ab", name=f"Ab{g}{p}")
                    nc.sync.dma_start(out=Ab, in_=d_vt[p, 128 * g: 128 * (g + 1), :])
                    Bb = rot.tile([128, VB], F32, tag="ab", name=f"Bb{g}{p}")
                    if g == 0:
                        nc.vector.memset(Bb[0:32, :], 0.0)
                        nc.sync.dma_start(out=Bb[1:128, :], in_=d_vt[p, 0:127, :])
                    else:
                        nc.sync.dma_start(out=Bb, in_=d_vt[p, 128 * g - 1: 128 * (g + 1) - 1, :])
                    cmpf = rot.tile([128, TDEC], F32, tag="work", name=f"cmpf{g}{p}")
                    nc.vector.tensor_tensor(out=cmpf, in0=Bb[:, 0:TDEC], in1=Ab[:, 0:TDEC],
                                            op=OP.is_lt)
                    nc.vector.tensor_tensor(out=dbu[:, (g * 4 + p) * TDEC:(g * 4 + p + 1) * TDEC],
                                            in0=cmpf, in1=band, op=OP.max)
            nc.sync.dma_start(out=outs["o_db"][:, :], in_=dbu)
            # out APs iterate (q, c, inner) to match the sbuf staging layout
            sim_ap = bass.AP(outs["o_sim"], 0, [[TDEC, 128], [2 * 128 * TDEC, NB], [128 * TDEC, 2], [1, TDEC]])
            nc.sync.dma_start(out=sim_ap, in_=simall.rearrange("p (c t) -> p c t", c=4))
            mask_ap = bass.AP(outs["o_mask"], 0, [[TDEC, 128], [2 * 128 * TDEC, NB], [128 * TDEC, 2], [1, TDEC]])
            nc.sync.dma_start(out=mask_ap, in_=m2all.rearrange("p (c t) -> p c t", c=4))
            lsm_ap = bass.AP(outs["o_lsm"], 0, [[VOCAB, 128], [8 * 128 * VOCAB, NB], [128 * VOCAB, 8], [1, VOCAB]])
            nc.sync.dma_start(out=lsm_ap, in_=lsmall.rearrange("p (c v) -> p c v", c=16))
            nc.sync.dma_start(out=outs["o_snls"][:, :], in_=snall)

    nc.finalize()
    return nc


# ----------------------------------------------------------------------------
# host: input prep
# ----------------------------------------------------------------------------

def _relu(x):
    return np.maximum(x, 0.0)


def _mlp2(v, w1, b1, w2, b2):
    h = _relu(v @ w1.T + b1)
    return _relu(h @ w2.T + b2)


def _prep_shared(params):
    p = {}
    aug = lambda w, b: np.ascontiguousarray(
        np.concatenate([np.asarray(w, np.float32).T, np.asarray(b, np.float32)[None, :]], 0))
    p["eWt"] = aug(params["enc_proj_w"], params["enc_proj_b"])
    p["qWt"] = aug(params["query_proj_w"], params["query_proj_b"])
    p["cWt"] = aug(params["ctc_proj_w"], params["ctc_proj_b"])
    for stk, tag in ((params["main"], "m"), (params["aux"], "a")):
        p[f"w0t_{tag}"] = np.ascontiguousarray(
            np.asarray(stk[0]["conv_w"], np.float32).transpose(2, 1, 0))
        p[f"wrt_{tag}"] = np.ascontiguousarray(np.stack(
            [np.asarray(stk[l]["conv_w"], np.float32).transpose(2, 1, 0) for l in range(1, NL)]))
        p[f"gnw_{tag}"] = np.ascontiguousarray(
            np.stack([np.asarray(stk[l]["gn_w"], np.float32) for l in range(NL)]))
        p[f"gnb_{tag}"] = np.ascontiguousarray(
            np.stack([np.asarray(stk[l]["gn_b"], np.float32) for l in range(NL)]))
    p["consts"] = np.array(
        [[10.0 * np.exp(np.float32(params["sim_w"])), np.float32(params["sim_b"]),
          1e-5, 0, 0, 0, 0, 0]], np.float32)
    ridx = np.empty((128, 8), np.float32)
    for pp in range(128):
        for g in range(8):
            ridx[pp, g] = 128 * g + pp
    p["rowidx"] = ridx
    return p


def _prep_core(c, text, spec, spkr_vec, gst_vec, text_lengths, spec_lengths,
               enc_input, params, shared):
    sl = slice(2 * c, 2 * c + 2)
    spec_c = np.asarray(spec[sl], np.float32)
    tl = np.asarray(text_lengths[sl]).astype(np.int64)
    sll = np.asarray(spec_lengths[sl]).astype(np.int64)
    smask = (np.arange(TDEC)[None, :] < sll[:, None]).astype(np.float32)
    d = dict(shared)
    xm = np.zeros((NB, MEL, TP), np.float32)
    xm[:, :, 1:1 + TDEC] = (spec_c * smask[:, :, None]).transpose(0, 2, 1)
    d["xm"] = xm
    d["spec_y"] = np.ascontiguousarray(spec_c.reshape(NB, 8, 128, MEL))
    tt = np.empty((NB, 257, TX), np.float32)
    tt[:, :256] = np.asarray(text[sl], np.float32).transpose(0, 2, 1)
    tt[:, 256] = 1.0
    d["textT"] = tt
    sm = np.zeros((NB, TP), np.float32)
    sm[:, 1:1 + TDEC] = smask
    d["smask"] = sm
    d["tmask"] = (np.arange(TX)[None, :] < tl[:, None]).astype(np.float32)
    eh = np.zeros((NB, VOCAB, TX), np.float32)
    ei = np.asarray(enc_input[sl]).astype(np.int64)
    for b in range(NB):
        eh[b, ei[b], np.arange(TX)] = 1.0
    d["ehotT"] = eh
    sv = np.asarray(spkr_vec[sl], np.float32)
    gv = np.asarray(gst_vec[sl], np.float32)
    for stk, tag in ((params["main"], "m"), (params["aux"], "a")):
        lb = np.empty((NL, NB, C), np.float32)
        for l in range(NL):
            pl = stk[l]
            lb[l] = (_mlp2(sv, np.asarray(pl["s1w"], np.float32), np.asarray(pl["s1b"], np.float32),
                           np.asarray(pl["s2w"], np.float32), np.asarray(pl["s2b"], np.float32))
                     + _mlp2(gv, np.asarray(pl["g1w"], np.float32), np.asarray(pl["g1b"], np.float32),
                             np.asarray(pl["g2w"], np.float32), np.asarray(pl["g2b"], np.float32)))
        d[f"lb_{tag}"] = lb
    return d


# ----------------------------------------------------------------------------
# host: post-processing
# ----------------------------------------------------------------------------

def _backtrack(D, t_x, t_y):
    """D [B,TX,TDEC] uint8; returns idx trajectories [B, TDEC] int64."""
    B = D.shape[0]
    bi = np.arange(B)
    index = (t_x - 1).astype(np.int64).copy()
    idx_traj = np.empty((B, TDEC), np.int64)
    for j in range(TDEC - 1, -1, -1):
        idx_traj[:, j] = index
        write = j < t_y
        move = (index != 0) & (D[bi, index, j] != 0)
        index = np.where(write & move, index - 1, index)
    return idx_traj


def _ctc_loss(log_probs_nt, targets, input_lengths, target_lengths, blank=0):
    """log_probs_nt [N, T, V] f32. Reference-faithful log-space CTC."""
    Nb, T, V = log_probs_nt.shape
    S = targets.shape[1]
    L = 2 * S + 1
    NEG = -1e9
    ext = np.full((Nb, L), blank, np.int64)
    ext[:, 1::2] = targets
    skip = np.concatenate([np.zeros((Nb, 2), bool),
                           (ext[:, 2:] != blank) & (ext[:, 2:] != ext[:, :-2])], axis=1)
    lp_ext = np.take_along_axis(log_probs_nt, np.broadcast_to(ext[:, None, :], (Nb, T, L)), axis=2)
    lp_ext = np.ascontiguousarray(lp_ext.transpose(1, 0, 2), dtype=np.float32)  # [T,N,L]
    alpha = np.full((Nb, L), NEG, np.float32)
    alpha[:, 0] = lp_ext[0, :, 0]
    alpha[:, 1] = lp_ext[0, :, 1]
    a2 = np.empty_like(alpha)
    a3 = np.empty_like(alpha)
    for t in range(1, T):
        a2[:, 0] = NEG
        a2[:, 1:] = alpha[:, :-1]
        a3[:, :2] = NEG
        a3[:, 2:] = np.where(skip[:, 2:], alpha[:, :-2], NEG)
        new = (np.logaddexp(np.logaddexp(alpha, a2), a3) + lp_ext[t]).astype(np.float32)
        upd = t < input_lengths
        alpha[upd] = new[upd]
    bi = np.arange(Nb)
    e1 = alpha[bi, 2 * target_lengths]
    e2 = alpha[bi, 2 * target_lengths - 1]
    loss = -np.logaddexp(e1, e2)
    return np.float32(np.mean(loss / target_lengths.astype(np.float32)))


_NC_CACHE = None


def kernel(text, spec, text_lengths, spec_lengths, spkr_vec, gst_vec, enc_input, params):
    global _NC_CACHE
    text = np.asarray(text)
    spec = np.asarray(spec)
    text_lengths = np.asarray(text_lengths)
    spec_lengths = np.asarray(spec_lengths)
    enc_input = np.asarray(enc_input)

    if _NC_CACHE is None:
        _NC_CACHE = build_nc()
    nc = _NC_CACHE

    shared = _prep_shared(params)
    in_maps = [
        _prep_core(c, text, spec, spkr_vec, gst_vec, text_lengths, spec_lengths,
                   enc_input, params, shared)
        for c in range(NCORES)
    ]
    res = run_bass_kernel_spmd(nc, in_maps, core_ids=list(range(NCORES)))
    results = res.results

    tl = text_lengths.astype(np.int64)
    sl = spec_lengths.astype(np.int64)
    sim = np.empty((N, TX, TDEC), np.float32)
    att_mask = np.empty((N, TX, TDEC), np.float32)
    lsm = np.empty((N, TDEC, VOCAB), np.float32)
    snls = np.empty((N,), np.float32)
    D1 = np.empty((N, TX, TDEC), np.uint8)
    D2 = np.empty((N, TX, TDEC), np.uint8)
    for c in range(NCORES):
        r = results[c]
        s2 = slice(2 * c, 2 * c + 2)
        sim[s2] = r["o_sim"].reshape(NB, TX, TDEC)
        att_mask[s2] = r["o_mask"].reshape(NB, TX, TDEC)
        lsm[s2] = r["o_lsm"].reshape(NB, TDEC, VOCAB)
        snls[s2] = r["o_snls"][:, 0]
        Db = r["o_db"].reshape(128, 2, 4, TDEC).transpose(2, 1, 0, 3).reshape(4, TX, TDEC)
        D1[2 * c] = Db[0]
        D1[2 * c + 1] = Db[1]
        D2[2 * c] = Db[2]
        D2[2 * c + 1] = Db[3]

    idx1 = _backtrack(D1, tl, sl)
    idx2 = _backtrack(D2, tl, sl)

    bi = np.arange(N)[:, None]
    jj = np.arange(TDEC)[None, :]
    wmask = jj < sl[:, None]

    attention = np.zeros((N, TX, TDEC), np.float32)
    bidx, jidx = np.nonzero(wmask)
    attention[bidx, idx1[bidx, jidx], jidx] = 1.0

    # losses
    denom = (tl * sl).astype(np.float32)
    path_sim = np.sum(sim[bi, idx1, jj] * wmask, axis=1, dtype=np.float64).astype(np.float32)
    icl = (snls - path_sim) / denom
    nll = np.float32(np.mean(icl))

    aux_sim = np.zeros((N,), np.float64)
    for dshift in (-1, 0, 1):
        ii = idx2 + dshift
        ok = wmask & (ii >= 0) & (ii < tl[:, None])
        aux_sim += np.sum(sim[bi, np.clip(ii, 0, TX - 1), jj] * ok, axis=1, dtype=np.float64)
    aux_l = (snls - aux_sim.astype(np.float32)) / denom * 0.5

    ctc = _ctc_loss(lsm, enc_input.astype(np.int64), sl, tl)
    att_loss = np.float32(nll + np.float32(np.mean(aux_l)) + ctc)

    return attention, att_loss, att_mask, np.float32(nll)


# revision 27
# speedup vs baseline: 1.0861x; 1.0861x over previous
"""Trainium2 Bass kernel for nn_Attention_40767829574453 (Glow-TTS style aligner).

Sharding: pure data parallelism, batch 16 -> 8 cores x 2 batches each.

Device (per core): both 6-layer conv stacks (masked conv1d as shifted matmuls +
groupnorm folded into a Relu activation + speaker/gst layer bias + residuals),
key/query projections + L2 normalization, similarity bmm, CTC softmax /
log-softmax, sim_ctc bmm (one-hot gather as matmul), softplus log-sigmoid
terms, masked reductions, BOTH monotonic-alignment forward DPs (rows of the DP
computed with hardware tensor_tensor_scan in a negated min/add form) and the
backtrack decision bits D = (i==j) | (v[i-1,j-1] > v[i,j-1]).

Host: tiny exact backtrack walks over the D bits, log-space CTC scan, scalar
loss assembly, one-hot attention build.
"""
import numpy as np

import concourse.bass as bass
import concourse.bacc as bacc
import concourse.mybir as mybir
import concourse.tile as tile
from concourse.bass_utils import run_bass_kernel_spmd
from concourse.masks import make_identity

F32 = mybir.dt.float32
U8 = mybir.dt.uint8
I32 = mybir.dt.int32
AF = mybir.ActivationFunctionType
OP = mybir.AluOpType
AX = mybir.AxisListType

N, TX, TDEC = 16, 256, 1024
MEL = 80
ATT_H = 128
VOCAB = 100
NL = 6
C = 256
NCORES = 8
NB = 2
BIGPOS = 1e9
TP = TDEC + 2
VB = TDEC + 1  # vtab block width (guard col + 1024)


# ----------------------------------------------------------------------------
# device program
# ----------------------------------------------------------------------------

def build_nc(phase=None):
    import os
    phase = phase or os.environ.get("KPHASE", "all")
    P = ["conv", "sim", "ctc", "scan", "d", "all"].index(phase)
    nc = bacc.Bacc("TRN2")
    dt = nc.dram_tensor
    ins = dict(
        xm=dt("xm", [NB, MEL, TP], F32, kind="ExternalInput"),
        spec_y=dt("spec_y", [NB, 8, 128, MEL], F32, kind="ExternalInput"),
        textT=dt("textT", [NB, 257, TX], F32, kind="ExternalInput"),
        eWt=dt("eWt", [257, ATT_H], F32, kind="ExternalInput"),
        qWt=dt("qWt", [257, ATT_H - 1], F32, kind="ExternalInput"),
        cWt=dt("cWt", [257, VOCAB], F32, kind="ExternalInput"),
        w0t_m=dt("w0t_m", [3, MEL, C], F32, kind="ExternalInput"),
        w0t_a=dt("w0t_a", [3, MEL, C], F32, kind="ExternalInput"),
        wrt_m=dt("wrt_m", [NL - 1, 3, C, C], F32, kind="ExternalInput"),
        wrt_a=dt("wrt_a", [NL - 1, 3, C, C], F32, kind="ExternalInput"),
        gnw_m=dt("gnw_m", [NL, C], F32, kind="ExternalInput"),
        gnb_m=dt("gnb_m", [NL, C], F32, kind="ExternalInput"),
        gnw_a=dt("gnw_a", [NL, C], F32, kind="ExternalInput"),
        gnb_a=dt("gnb_a", [NL, C], F32, kind="ExternalInput"),
        lb_m=dt("lb_m", [NL, NB, C], F32, kind="ExternalInput"),
        lb_a=dt("lb_a", [NL, NB, C], F32, kind="ExternalInput"),
        smask=dt("smask", [NB, TP], F32, kind="ExternalInput"),
        tmask=dt("tmask", [NB, TX], F32, kind="ExternalInput"),
        ehotT=dt("ehotT", [NB, VOCAB, TX], F32, kind="ExternalInput"),
        consts=dt("consts", [1, 8], F32, kind="ExternalInput"),
        rowidx=dt("rowidx", [128, 8], F32, kind="ExternalInput"),
    )
    outs = dict(
        o_sim=dt("o_sim", [NB, 2, 128, TDEC], F32, kind="ExternalOutput"),
        o_lsm=dt("o_lsm", [NB, 8, 128, VOCAB], F32, kind="ExternalOutput"),
        o_db=dt("o_db", [128, 8 * TDEC], U8, kind="ExternalOutput"),
        o_mask=dt("o_mask", [NB, 2, 128, TDEC], F32, kind="ExternalOutput"),
        o_snls=dt("o_snls", [NB, 1], F32, kind="ExternalOutput"),
    )

    with tile.TileContext(nc) as tc:
        with (
            tc.tile_pool(name="per", bufs=1) as per,      # persistent, unique tags
            tc.tile_pool(name="xp", bufs=5) as xp,        # conv activations
            tc.tile_pool(name="wt", bufs=9) as wtp,      # conv weights
            tc.tile_pool(name="st", bufs=6) as stp,       # [128,1]-ish stats
            tc.tile_pool(name="rot", bufs=3) as rot,      # rotating [128,1024] work
            tc.tile_pool(name="cps", bufs=4, space="PSUM") as psp,   # [128,512]
            tc.tile_pool(name="sps", bufs=3, space="PSUM") as psp2,  # small psums
        ):
            ident = per.tile([128, 128], F32, tag="ident")
            make_identity(nc, ident)

            ones_col = per.tile([128, 1], F32, tag="ones_col")
            nc.vector.memset(ones_col, 1.0)
            ones_row = per.tile([1, 128], F32, tag="ones_row")
            nc.vector.memset(ones_row, 1.0)
            bigpos = per.tile([128, TDEC], F32, tag="bigpos")
            nc.vector.memset(bigpos, BIGPOS)
            jio = per.tile([128, TDEC], F32, tag="jio")
            nc.gpsimd.iota(jio, pattern=[[1, TDEC]], base=0, channel_multiplier=0, allow_small_or_imprecise_dtypes=True)
            csts = per.tile([128, 8], F32, tag="csts")
            nc.gpsimd.dma_start(out=csts, in_=bass.AP(ins["consts"], 0, [[0, 128], [1, 8]]))
            sc10 = csts[:, 0:1]
            smb = csts[:, 1:2]
            epsc = csts[:, 2:3]
            rix = per.tile([128, 8], F32, tag="rix")
            nc.sync.dma_start(out=rix, in_=ins["rowidx"][:, :])

            gw_m = per.tile([128, NL * 2], F32, tag="gw_m")
            gb_m = per.tile([128, NL * 2], F32, tag="gb_m")
            gw_a = per.tile([128, NL * 2], F32, tag="gw_a")
            gb_a = per.tile([128, NL * 2], F32, tag="gb_a")
            lbm = per.tile([128, NL * NB * 2], F32, tag="lbm")
            lba = per.tile([128, NL * NB * 2], F32, tag="lba")
            # single strided DMA each: [128 part, (l, ct)] / [128, (l, b, ct)]
            for t_sb, t_dram in ((gw_m, "gnw_m"), (gb_m, "gnb_m"), (gw_a, "gnw_a"), (gb_a, "gnb_a")):
                nc.sync.dma_start(out=t_sb[:, :],
                                  in_=bass.AP(ins[t_dram], 0, [[1, 128], [C, NL], [128, 2]]))
            for t_sb, t_dram in ((lbm, "lb_m"), (lba, "lb_a")):
                nc.sync.dma_start(out=t_sb[:, :],
                                  in_=bass.AP(ins[t_dram], 0, [[1, 128], [NB * C, NL], [C, NB], [128, 2]]))

            maskbc = []
            for b in range(NB):
                mb = per.tile([128, TDEC], F32, tag=f"maskbc{b}")
                nc.gpsimd.dma_start(
                    out=mb, in_=bass.AP(ins["smask"], b * TP + 1, [[0, 128], [1, TDEC]]))
                maskbc.append(mb)
            tmcol = per.tile([128, NB * 2], F32, tag="tmcol")
            nc.sync.dma_start(out=tmcol[:, :],
                              in_=bass.AP(ins["tmask"], 0, [[1, 128], [TX, NB], [128, 2]]))

            d_val = nc.dram_tensor("d_val", [4, TX, TDEC], F32)
            d_vt = nc.dram_tensor("d_vt", [4, TX, VB], F32)
            NSLOT = 6
            slotring = per.tile([128, NSLOT * VB], F32, tag="slotring")
            valring = per.tile([128, NSLOT * TDEC], F32, tag="valring")
            simall = per.tile([128, 4 * TDEC], F32, tag="simall")
            m2all = per.tile([128, 4 * TDEC], F32, tag="m2all")
            lsmall = per.tile([128, 16 * VOCAB], F32, tag="lsmall")
            snall = per.tile([1, NB], F32, tag="snall")
            for s in range(NSLOT):
                nc.vector.memset(slotring[0:4, s * VB: s * VB + 1], BIGPOS)


            def conv_stack(b, aux):
                w0 = ins["w0t_a" if aux else "w0t_m"]
                wr = ins["wrt_a" if aux else "wrt_m"]
                gw = gw_a if aux else gw_m
                gb = gb_a if aux else gb_m
                lb = lba if aux else lbm
                x0 = xp.tile([128, TP], F32, tag="x")
                nc.sync.dma_start(out=x0[:MEL, :], in_=ins["xm"][b])
                x = [x0]
                for l in range(NL):
                    cin_tiles = 1 if l == 0 else 2
                    kdim = MEL if l == 0 else 128
                    wts = []
                    for k in range(3):
                        for cit in range(cin_tiles):
                            w = wtp.tile([128, C], F32, tag="wt")
                            if l == 0:
                                nc.sync.dma_start(out=w[:MEL, :], in_=w0[k])
                            else:
                                nc.sync.dma_start(out=w, in_=wr[l - 1, k, cit * 128:(cit + 1) * 128, :])
                            wts.append(w)
                    xn = []
                    for ct in range(2):
                        stats = stp.tile([128, 2, 6], F32, tag="bnst")
                        pss = []
                        for lc in range(2):
                            ps = psp.tile([128, 512], F32, tag="cps")
                            mm = 0
                            for k in range(3):
                                for cit in range(cin_tiles):
                                    nc.tensor.matmul(
                                        ps,
                                        wts[k * cin_tiles + cit][:kdim, ct * 128:(ct + 1) * 128],
                                        x[cit][:kdim, lc * 512 + k: lc * 512 + k + 512],
                                        start=(mm == 0), stop=(mm == 3 * cin_tiles - 1))
                                    mm += 1
                            nc.vector.bn_stats(out=stats[:, lc, :], in_=ps[:, :])
                            pss.append(ps)
                        mv = stp.tile([128, 2], F32, tag="bnmv")
                        nc.vector.bn_aggr(out=mv, in_=stats)
                        sd = stp.tile([128, 1], F32, tag="sd")
                        nc.scalar.activation(out=sd, in_=mv[:, 1:2], func=AF.Sqrt, bias=epsc)
                        rs = stp.tile([128, 1], F32, tag="rs")
                        nc.vector.reciprocal(rs, sd)
                        ga = stp.tile([128, 1], F32, tag="ga")
                        nc.vector.tensor_mul(ga, rs, gw[:, l * 2 + ct: l * 2 + ct + 1])
                        gbb = stp.tile([128, 1], F32, tag="gbb")
                        nc.vector.tensor_mul(gbb, mv[:, 0:1], ga)
                        nc.vector.tensor_sub(gbb, gb[:, l * 2 + ct: l * 2 + ct + 1], gbb)
                        xt_ = xp.tile([128, TP], F32, tag="x")
                        for lc in range(2):
                            nc.scalar.activation(out=xt_[:, 1 + lc * 512: 1 + (lc + 1) * 512],
                                                 in_=pss[lc], func=AF.Relu, scale=ga, bias=gbb)
                        lcol = lb[:, (l * NB + b) * 2 + ct: (l * NB + b) * 2 + ct + 1]
                        if l % 2 == 1:
                            # (lbias + y) + x_prev, then mask
                            nc.vector.scalar_tensor_tensor(
                                out=xt_[:, 1:1 + TDEC], in0=xt_[:, 1:1 + TDEC], scalar=lcol,
                                in1=x[ct][:, 1:1 + TDEC], op0=OP.add, op1=OP.add)
                            if l < NL - 1:
                                nc.vector.tensor_mul(xt_[:, 1:1 + TDEC], xt_[:, 1:1 + TDEC],
                                                     maskbc[b])
                        else:
                            # (lbias + y) * mask in one op
                            if l < NL - 1:
                                nc.vector.scalar_tensor_tensor(
                                    out=xt_[:, 1:1 + TDEC], in0=xt_[:, 1:1 + TDEC], scalar=lcol,
                                    in1=maskbc[b], op0=OP.add, op1=OP.mult)
                            else:
                                nc.vector.tensor_scalar_add(xt_[:, 1:1 + TDEC],
                                                            xt_[:, 1:1 + TDEC], lcol)
                        nc.vector.memset(xt_[:, 0:1], 0.0)
                        nc.vector.memset(xt_[:, TP - 1: TP], 0.0)
                        xn.append(xt_)
                    x = xn
                return x

            def normalize_rows(t, n_free):
                scr = rot.tile([128, n_free], F32, tag="sw")
                sq = stp.tile([128, 1], F32, tag="nsq")
                nc.vector.tensor_mul(scr, t[:, :n_free], t[:, :n_free])
                nc.vector.reduce_sum(sq, scr, axis=AX.X)
                sd = stp.tile([128, 1], F32, tag="nsd")
                nc.scalar.activation(out=sd, in_=sq, func=AF.Sqrt)
                rr = stp.tile([128, 1], F32, tag="nrr")
                nc.vector.reciprocal(rr, sd)
                nc.vector.tensor_scalar_mul(t[:, :n_free], t[:, :n_free], rr)

            def stage_val(val, b, m, xt):
                p = 2 * m + b
                nc.sync.dma_start(out=d_val[p, xt * 128:(xt + 1) * 128, :], in_=val)

            # ---------------- main stacks, key/query, similarity ----------------
            m2_tiles = {}
            for b in range(NB):
                x6 = conv_stack(b, aux=False)
                if P < 1:
                    continue
                SUB = int(os.environ.get("KSUB", "9"))

                # key
                tta = [per.tile([128, TX], F32, tag=f"tt{i}", name=f"tt{i}") for i in range(2)]
                ttb = per.tile([1, TX], F32, tag="ttb")
                for i in range(2):
                    nc.sync.dma_start(out=tta[i], in_=ins["textT"][b, i * 128:(i + 1) * 128, :])
                nc.sync.dma_start(out=ttb, in_=ins["textT"][b, 256:257, :])
                if b == 0:
                    eW = [per.tile([128, ATT_H], F32, tag=f"eW{i}", name=f"eW{i}") for i in range(2)]
                    eWb = per.tile([1, ATT_H], F32, tag="eWb")
                    for i in range(2):
                        nc.sync.dma_start(out=eW[i], in_=ins["eWt"][i * 128:(i + 1) * 128, :])
                    nc.sync.dma_start(out=eWb, in_=ins["eWt"][256:257, :])
                    qW = [per.tile([128, ATT_H - 1], F32, tag=f"qW{i}", name=f"qW{i}") for i in range(2)]
                    qWb = per.tile([1, ATT_H - 1], F32, tag="qWb")
                    for i in range(2):
                        nc.sync.dma_start(out=qW[i], in_=ins["qWt"][i * 128:(i + 1) * 128, :])
                    nc.sync.dma_start(out=qWb, in_=ins["qWt"][256:257, :])
                keyT = per.tile([128, TX], F32, tag=f"keyT{b}")
                nc.vector.memset(keyT, 0.0)
                for xt in range(2):
                    if SUB < 1:
                        continue
                    pk = psp2.tile([128, ATT_H], F32, tag="sp")
                    for i in range(2):
                        nc.tensor.matmul(pk, tta[i][:, xt * 128:(xt + 1) * 128], eW[i],
                                         start=(i == 0), stop=False)
                    nc.tensor.matmul(pk, ttb[:, xt * 128:(xt + 1) * 128], eWb,
                                     start=False, stop=True)
                    kk = rot.tile([128, ATT_H], F32, tag="sw")
                    nc.scalar.activation(out=kk, in_=pk, func=AF.Copy)
                    if SUB >= 2:
                        normalize_rows(kk, ATT_H)
                    if SUB >= 3:
                        pt = psp2.tile([128, 128], F32, tag="sp")
                        nc.tensor.transpose(pt, kk, ident)
                        nc.scalar.activation(out=keyT[:, xt * 128:(xt + 1) * 128], in_=pt, func=AF.Copy)

                if SUB < 4:
                    continue
                # query
                qT = per.tile([128, TDEC], F32, tag="qT")
                for yt in range(8):
                    pq = psp2.tile([128, ATT_H - 1], F32, tag="sp")
                    for i in range(2):
                        nc.tensor.matmul(pq, x6[i][:, 1 + yt * 128: 1 + (yt + 1) * 128], qW[i],
                                         start=(i == 0), stop=False)
                    nc.tensor.matmul(pq, ones_row, qWb,
                                     start=False, stop=True)
                    qq = rot.tile([128, 128], F32, tag="sw")
                    nc.scalar.activation(out=qq[:, 0:ATT_H - 1], in_=pq, func=AF.Copy)
                    sy = rot.tile([128, MEL], F32, tag="sw")
                    nc.sync.dma_start(out=sy, in_=ins["spec_y"][b, yt])
                    en = stp.tile([128, 1], F32, tag="en")
                    nc.vector.reduce_sum(en, sy, axis=AX.X)
                    nc.vector.tensor_scalar_mul(qq[:, ATT_H - 1: ATT_H], en, 1.0 / MEL)
                    normalize_rows(qq, ATT_H)
                    pt = psp2.tile([128, 128], F32, tag="sp")
                    nc.tensor.transpose(pt, qq, ident)
                    nc.scalar.activation(out=qT[:, yt * 128:(yt + 1) * 128], in_=pt, func=AF.Copy)

                if SUB < 5:
                    continue
                # similarity + masked softplus sums + val1
                snps = []
                for xt in range(2):
                    bx = b * 2 + xt
                    ssim = simall[:, bx * TDEC:(bx + 1) * TDEC]
                    for yc in range(2):
                        psim = psp.tile([128, 512], F32, tag="cps")
                        nc.tensor.matmul(psim, keyT[:, xt * 128:(xt + 1) * 128],
                                         qT[:, yc * 512:(yc + 1) * 512], start=True, stop=True)
                        nc.scalar.activation(out=ssim[:, yc * 512:(yc + 1) * 512], in_=psim,
                                             func=AF.Identity, scale=sc10, bias=smb)
                    m2 = m2all[:, bx * TDEC:(bx + 1) * TDEC]
                    nc.vector.tensor_scalar_mul(m2, maskbc[b], tmcol[:, b * 2 + xt: b * 2 + xt + 1])
                    m2_tiles[(b, xt)] = m2
                    # nlsn = softplus(sim) = relu(sim) + ln(1 + exp(-|sim|))
                    ab = rot.tile([128, TDEC], F32, tag="work")
                    nc.scalar.activation(out=ab, in_=ssim, func=AF.Abs)
                    nc.scalar.activation(out=ab, in_=ab, func=AF.Exp, scale=-1.0)
                    nc.scalar.activation(out=ab, in_=ab, func=AF.Ln, bias=ones_col)
                    nlsn = rot.tile([128, TDEC], F32, tag="work")
                    nc.scalar.activation(out=nlsn, in_=ssim, func=AF.Relu)
                    nc.vector.tensor_add(nlsn, nlsn, ab)
                    scr = rot.tile([128, TDEC], F32, tag="work")
                    snp = stp.tile([128, 1], F32, tag=f"snp{xt}")
                    nc.vector.tensor_mul(scr, nlsn, m2)
                    nc.vector.reduce_sum(snp, scr, axis=AX.X)
                    snps.append(snp)
                    # softplus(-sim) = softplus(sim) - sim
                    lsn = rot.tile([128, TDEC], F32, tag="work")
                    nc.vector.tensor_sub(lsn, nlsn, ssim)
                    val = rot.tile([128, TDEC], F32, tag="work")
                    nc.vector.tensor_mul(val, lsn, m2)
                    stage_val(val, b, 0, xt)
                sn2 = stp.tile([128, 1], F32, tag="sn2")
                nc.vector.tensor_add(sn2, snps[0], snps[1])
                psn = psp2.tile([1, 1], F32, tag="sp")
                nc.tensor.matmul(psn, sn2, ones_col, start=True, stop=True)
                nc.scalar.activation(out=snall[:, b:b + 1], in_=psn, func=AF.Copy)

            # ---------------- aux stacks, ctc, sim_ctc, val2 ----------------
            for b in range(NB if P >= 2 else 0):
                x6a = conv_stack(b, aux=True)
                if b == 0:
                    cW = [per.tile([128, VOCAB], F32, tag=f"cW{i}", name=f"cW{i}") for i in range(2)]
                    cWb = per.tile([1, VOCAB], F32, tag="cWb")
                    for i in range(2):
                        nc.sync.dma_start(out=cW[i], in_=ins["cWt"][i * 128:(i + 1) * 128, :])
                    nc.sync.dma_start(out=cWb, in_=ins["cWt"][256:257, :])
                qTc = per.tile([128, TDEC], F32, tag="qTc")
                for yt in range(8):
                    pc = psp2.tile([128, VOCAB], F32, tag="sp")
                    for i in range(2):
                        nc.tensor.matmul(pc, x6a[i][:, 1 + yt * 128: 1 + (yt + 1) * 128], cW[i],
                                         start=(i == 0), stop=False)
                    nc.tensor.matmul(pc, ones_row, cWb,
                                     start=False, stop=True)
                    mx = stp.tile([128, 1], F32, tag="mx")
                    nc.vector.reduce_max(mx, pc, axis=AX.X)
                    nmx = stp.tile([128, 1], F32, tag="nmx")
                    nc.vector.tensor_scalar_mul(nmx, mx, -1.0)
                    exb = rot.tile([128, VOCAB], F32, tag="sw")
                    nc.scalar.activation(out=exb, in_=pc, func=AF.Exp, bias=nmx)
                    s = stp.tile([128, 1], F32, tag="s")
                    nc.vector.reduce_sum(s, exb, axis=AX.X)
                    lns = stp.tile([128, 1], F32, tag="lns")
                    nc.scalar.activation(out=lns, in_=s, func=AF.Ln)
                    r = stp.tile([128, 1], F32, tag="r")
                    nc.vector.reciprocal(r, s)
                    qsb = rot.tile([128, VOCAB], F32, tag="sw")
                    nc.vector.tensor_scalar_mul(qsb, exb, r)
                    nb_ = stp.tile([128, 1], F32, tag="nb_")
                    nc.vector.tensor_add(nb_, mx, lns)
                    nc.vector.tensor_scalar_mul(nb_, nb_, -1.0)
                    lidx = b * 8 + yt
                    nc.scalar.activation(out=lsmall[:, lidx * VOCAB:(lidx + 1) * VOCAB],
                                         in_=pc, func=AF.Identity, bias=nb_)
                    ptc = psp2.tile([128, 128], F32, tag="sp")
                    nc.tensor.transpose(ptc[:VOCAB, :], qsb, ident)
                    nc.scalar.activation(out=qTc[:VOCAB, yt * 128:(yt + 1) * 128],
                                         in_=ptc[:VOCAB, :], func=AF.Copy)
                eh = per.tile([128, TX], F32, tag="eh")
                nc.sync.dma_start(out=eh[:VOCAB, :], in_=ins["ehotT"][b])
                for xt in range(2):
                    scs = rot.tile([128, TDEC], F32, tag="work")
                    for yc in range(2):
                        psc = psp.tile([128, 512], F32, tag="cps")
                        nc.tensor.matmul(psc, eh[:VOCAB, xt * 128:(xt + 1) * 128],
                                         qTc[:VOCAB, yc * 512:(yc + 1) * 512],
                                         start=True, stop=True)
                        nc.scalar.activation(out=scs[:, yc * 512:(yc + 1) * 512], in_=psc,
                                             func=AF.Copy)
                    # softplus(-simc) = relu(-simc) + ln(1 + exp(-|simc|))
                    ab2 = rot.tile([128, TDEC], F32, tag="work")
                    nc.scalar.activation(out=ab2, in_=scs, func=AF.Abs)
                    nc.scalar.activation(out=ab2, in_=ab2, func=AF.Exp, scale=-1.0)
                    nc.scalar.activation(out=ab2, in_=ab2, func=AF.Ln, bias=ones_col)
                    lsn2 = rot.tile([128, TDEC], F32, tag="work")
                    nc.scalar.activation(out=lsn2, in_=scs, func=AF.Relu, scale=-1.0)
                    nc.vector.tensor_add(lsn2, lsn2, ab2)
                    val2 = rot.tile([128, TDEC], F32, tag="work")
                    nc.vector.tensor_mul(val2, lsn2, m2_tiles[(b, xt)])
                    stage_val(val2, b, 1, xt)

            # ---------------- MAS forward scans ----------------

            # rows beyond 223 can never be touched: text_lengths < 225 and the DP
            # flows strictly downward in i, so cap the scan row count.
            TX_SCAN = 224
            for i in range(TX_SCAN if P >= 3 else 0):
                s = i % NSLOT
                nc.sync.dma_start(out=valring[0:4, s * TDEC:(s + 1) * TDEC],
                                  in_=d_val[0:4, i, :])
                if i == 0:
                    data0 = bigpos[0:4, :]
                    init = 0.0
                else:
                    sp_ = (i - 1) % NSLOT
                    data0 = slotring[0:4, sp_ * VB: sp_ * VB + TDEC]
                    init = BIGPOS
                nc.vector.tensor_tensor_scan(
                    out=slotring[0:4, s * VB + 1: s * VB + 1 + TDEC],
                    data0=data0,
                    data1=valring[0:4, s * TDEC:(s + 1) * TDEC],
                    initial=init, op0=OP.min, op1=OP.add)
                nc.sync.dma_start(out=d_vt[0:4, i, :], in_=slotring[0:4, s * VB: (s + 1) * VB])

            # ---------------- D bits ----------------
            dbu = per.tile([128, 8 * TDEC], U8, tag="dbu")
            for g in range(2 if P >= 4 else 0):
                band = rot.tile([128, TDEC], F32, tag="work", name=f"band{g}")
                nc.vector.tensor_scalar(band, jio, rix[:, g:g + 1], None, op0=OP.is_equal)
                for p in range(4):
                    Ab = rot.tile([128, VB], F32, tag="ab", name=f"Ab{g}{p}")
                    nc.sync.dma_start(out=Ab, in_=d_vt[p, 128 * g: 128 * (g + 1), :])
                    Bb = rot.tile([128, VB], F32, tag="ab", name=f"Bb{g}{p}")
                    if g == 0:
                        nc.vector.memset(Bb[0:32, :], 0.0)
                        nc.sync.dma_start(out=Bb[1:128, :], in_=d_vt[p, 0:127, :])
                    else:
                        nc.sync.dma_start(out=Bb, in_=d_vt[p, 128 * g - 1: 128 * (g + 1) - 1, :])
                    cmpf = rot.tile([128, TDEC], F32, tag="work", name=f"cmpf{g}{p}")
                    nc.vector.tensor_tensor(out=cmpf, in0=Bb[:, 0:TDEC], in1=Ab[:, 0:TDEC],
                                            op=OP.is_lt)
                    nc.vector.tensor_tensor(out=dbu[:, (g * 4 + p) * TDEC:(g * 4 + p + 1) * TDEC],
                                            in0=cmpf, in1=band, op=OP.max)
            nc.sync.dma_start(out=outs["o_db"][:, :], in_=dbu)
            # out APs iterate (q, c, inner) to match the sbuf staging layout
            sim_ap = bass.AP(outs["o_sim"], 0, [[TDEC, 128], [2 * 128 * TDEC, NB], [128 * TDEC, 2], [1, TDEC]])
            nc.sync.dma_start(out=sim_ap, in_=simall.rearrange("p (c t) -> p c t", c=4))
            mask_ap = bass.AP(outs["o_mask"], 0, [[TDEC, 128], [2 * 128 * TDEC, NB], [128 * TDEC, 2], [1, TDEC]])
            nc.sync.dma_start(out=mask_ap, in_=m2all.rearrange("p (c t) -> p c t", c=4))
            lsm_ap = bass.AP(outs["o_lsm"], 0, [[VOCAB, 128], [8 * 128 * VOCAB, NB], [128 * VOCAB, 8], [1, VOCAB]])
            nc.sync.dma_start(out=lsm_ap, in_=lsmall.rearrange("p (c v) -> p c v", c=16))
            nc.sync.dma_start(out=outs["o_snls"][:, :], in_=snall)

    nc.finalize()
    return nc


# ----------------------------------------------------------------------------
# host: input prep
# ----------------------------------------------------------------------------

def _relu(x):
    return np.maximum(x, 0.0)


def _mlp2(v, w1, b1, w2, b2):
    h = _relu(v @ w1.T + b1)
    return _relu(h @ w2.T + b2)


def _prep_shared(params):
    p = {}
    aug = lambda w, b: np.ascontiguousarray(
        np.concatenate([np.asarray(w, np.float32).T, np.asarray(b, np.float32)[None, :]], 0))
    p["eWt"] = aug(params["enc_proj_w"], params["enc_proj_b"])
    p["qWt"] = aug(params["query_proj_w"], params["query_proj_b"])
    p["cWt"] = aug(params["ctc_proj_w"], params["ctc_proj_b"])
    for stk, tag in ((params["main"], "m"), (params["aux"], "a")):
        p[f"w0t_{tag}"] = np.ascontiguousarray(
            np.asarray(stk[0]["conv_w"], np.float32).transpose(2, 1, 0))
        p[f"wrt_{tag}"] = np.ascontiguousarray(np.stack(
            [np.asarray(stk[l]["conv_w"], np.float32).transpose(2, 1, 0) for l in range(1, NL)]))
        p[f"gnw_{tag}"] = np.ascontiguousarray(
            np.stack([np.asarray(stk[l]["gn_w"], np.float32) for l in range(NL)]))
        p[f"gnb_{tag}"] = np.ascontiguousarray(
            np.stack([np.asarray(stk[l]["gn_b"], np.float32) for l in range(NL)]))
    p["consts"] = np.array(
        [[10.0 * np.exp(np.float32(params["sim_w"])), np.float32(params["sim_b"]),
          1e-5, 0, 0, 0, 0, 0]], np.float32)
    ridx = np.empty((128, 8), np.float32)
    for pp in range(128):
        for g in range(8):
            ridx[pp, g] = 128 * g + pp
    p["rowidx"] = ridx
    return p


def _prep_core(c, text, spec, spkr_vec, gst_vec, text_lengths, spec_lengths,
               enc_input, params, shared):
    sl = slice(2 * c, 2 * c + 2)
    spec_c = np.asarray(spec[sl], np.float32)
    tl = np.asarray(text_lengths[sl]).astype(np.int64)
    sll = np.asarray(spec_lengths[sl]).astype(np.int64)
    smask = (np.arange(TDEC)[None, :] < sll[:, None]).astype(np.float32)
    d = dict(shared)
    xm = np.zeros((NB, MEL, TP), np.float32)
    xm[:, :, 1:1 + TDEC] = (spec_c * smask[:, :, None]).transpose(0, 2, 1)
    d["xm"] = xm
    d["spec_y"] = np.ascontiguousarray(spec_c.reshape(NB, 8, 128, MEL))
    tt = np.empty((NB, 257, TX), np.float32)
    tt[:, :256] = np.asarray(text[sl], np.float32).transpose(0, 2, 1)
    tt[:, 256] = 1.0
    d["textT"] = tt
    sm = np.zeros((NB, TP), np.float32)
    sm[:, 1:1 + TDEC] = smask
    d["smask"] = sm
    d["tmask"] = (np.arange(TX)[None, :] < tl[:, None]).astype(np.float32)
    eh = np.zeros((NB, VOCAB, TX), np.float32)
    ei = np.asarray(enc_input[sl]).astype(np.int64)
    for b in range(NB):
        eh[b, ei[b], np.arange(TX)] = 1.0
    d["ehotT"] = eh
    sv = np.asarray(spkr_vec[sl], np.float32)
    gv = np.asarray(gst_vec[sl], np.float32)
    for stk, tag in ((params["main"], "m"), (params["aux"], "a")):
        lb = np.empty((NL, NB, C), np.float32)
        for l in range(NL):
            pl = stk[l]
            lb[l] = (_mlp2(sv, np.asarray(pl["s1w"], np.float32), np.asarray(pl["s1b"], np.float32),
                           np.asarray(pl["s2w"], np.float32), np.asarray(pl["s2b"], np.float32))
                     + _mlp2(gv, np.asarray(pl["g1w"], np.float32), np.asarray(pl["g1b"], np.float32),
                             np.asarray(pl["g2w"], np.float32), np.asarray(pl["g2b"], np.float32)))
        d[f"lb_{tag}"] = lb
    return d


# ----------------------------------------------------------------------------
# host: post-processing
# ----------------------------------------------------------------------------

def _backtrack(D, t_x, t_y):
    """D [B,TX,TDEC] uint8; returns idx trajectories [B, TDEC] int64."""
    B = D.shape[0]
    bi = np.arange(B)
    index = (t_x - 1).astype(np.int64).copy()
    idx_traj = np.empty((B, TDEC), np.int64)
    for j in range(TDEC - 1, -1, -1):
        idx_traj[:, j] = index
        write = j < t_y
        move = (index != 0) & (D[bi, index, j] != 0)
        index = np.where(write & move, index - 1, index)
    return idx_traj


def _ctc_loss(log_probs_nt, targets, input_lengths, target_lengths, blank=0):
    """log_probs_nt [N, T, V] f32. Reference-faithful log-space CTC."""
    Nb, T, V = log_probs_nt.shape
    S = targets.shape[1]
    L = 2 * S + 1
    NEG = -1e9
    ext = np.full((Nb, L), blank, np.int64)
    ext[:, 1::2] = targets
    skip = np.concatenate([np.zeros((Nb, 2), bool),
                           (ext[:, 2:] != blank) & (ext[:, 2:] != ext[:, :-2])], axis=1)
    lp_ext = np.take_along_axis(log_probs_nt, np.broadcast_to(ext[:, None, :], (Nb, T, L)), axis=2)
    lp_ext = np.ascontiguousarray(lp_ext.transpose(1, 0, 2), dtype=np.float32)  # [T,N,L]
    alpha = np.full((Nb, L), NEG, np.float32)
    alpha[:, 0] = lp_ext[0, :, 0]
    alpha[:, 1] = lp_ext[0, :, 1]
    a2 = np.empty_like(alpha)
    a3 = np.empty_like(alpha)
    for t in range(1, T):
        a2[:, 0] = NEG
        a2[:, 1:] = alpha[:, :-1]
        a3[:, :2] = NEG
        a3[:, 2:] = np.where(skip[:, 2:], alpha[:, :-2], NEG)
        new = (np.logaddexp(np.logaddexp(alpha, a2), a3) + lp_ext[t]).astype(np.float32)
        upd = t < input_lengths
        alpha[upd] = new[upd]
    bi = np.arange(Nb)
    e1 = alpha[bi, 2 * target_lengths]
    e2 = alpha[bi, 2 * target_lengths - 1]
    loss = -np.logaddexp(e1, e2)
    return np.float32(np.mean(loss / target_lengths.astype(np.float32)))


_NC_CACHE = None


def kernel(text, spec, text_lengths, spec_lengths, spkr_vec, gst_vec, enc_input, params):
    global _NC_CACHE
    text = np.asarray(text)
    spec = np.asarray(spec)
    text_lengths = np.asarray(text_lengths)
    spec_lengths = np.asarray(spec_lengths)
    enc_input = np.asarray(enc_input)

    if _NC_CACHE is None:
        _NC_CACHE = build_nc()
    nc = _NC_CACHE

    shared = _prep_shared(params)
    in_maps = [
        _prep_core(c, text, spec, spkr_vec, gst_vec, text_lengths, spec_lengths,
                   enc_input, params, shared)
        for c in range(NCORES)
    ]
    res = run_bass_kernel_spmd(nc, in_maps, core_ids=list(range(NCORES)))
    results = res.results

    tl = text_lengths.astype(np.int64)
    sl = spec_lengths.astype(np.int64)
    sim = np.empty((N, TX, TDEC), np.float32)
    att_mask = np.empty((N, TX, TDEC), np.float32)
    lsm = np.empty((N, TDEC, VOCAB), np.float32)
    snls = np.empty((N,), np.float32)
    D1 = np.empty((N, TX, TDEC), np.uint8)
    D2 = np.empty((N, TX, TDEC), np.uint8)
    for c in range(NCORES):
        r = results[c]
        s2 = slice(2 * c, 2 * c + 2)
        sim[s2] = r["o_sim"].reshape(NB, TX, TDEC)
        att_mask[s2] = r["o_mask"].reshape(NB, TX, TDEC)
        lsm[s2] = r["o_lsm"].reshape(NB, TDEC, VOCAB)
        snls[s2] = r["o_snls"][:, 0]
        Db = r["o_db"].reshape(128, 2, 4, TDEC).transpose(2, 1, 0, 3).reshape(4, TX, TDEC)
        D1[2 * c] = Db[0]
        D1[2 * c + 1] = Db[1]
        D2[2 * c] = Db[2]
        D2[2 * c + 1] = Db[3]

    idx1 = _backtrack(D1, tl, sl)
    idx2 = _backtrack(D2, tl, sl)

    bi = np.arange(N)[:, None]
    jj = np.arange(TDEC)[None, :]
    wmask = jj < sl[:, None]

    attention = np.zeros((N, TX, TDEC), np.float32)
    bidx, jidx = np.nonzero(wmask)
    attention[bidx, idx1[bidx, jidx], jidx] = 1.0

    # losses
    denom = (tl * sl).astype(np.float32)
    path_sim = np.sum(sim[bi, idx1, jj] * wmask, axis=1, dtype=np.float64).astype(np.float32)
    icl = (snls - path_sim) / denom
    nll = np.float32(np.mean(icl))

    aux_sim = np.zeros((N,), np.float64)
    for dshift in (-1, 0, 1):
        ii = idx2 + dshift
        ok = wmask & (ii >= 0) & (ii < tl[:, None])
        aux_sim += np.sum(sim[bi, np.clip(ii, 0, TX - 1), jj] * ok, axis=1, dtype=np.float64)
    aux_l = (snls - aux_sim.astype(np.float32)) / denom * 0.5

    ctc = _ctc_loss(lsm, enc_input.astype(np.int64), sl, tl)
    att_loss = np.float32(nll + np.float32(np.mean(aux_l)) + ctc)

    return attention, att_loss, att_mask, np.float32(nll)


# revision 33
# speedup vs baseline: 1.0951x; 1.0083x over previous
"""Trainium2 Bass kernel for nn_Attention_40767829574453 (Glow-TTS style aligner).

Sharding: pure data parallelism, batch 16 -> 8 cores x 2 batches each.

Device (per core): both 6-layer conv stacks (masked conv1d as shifted matmuls +
groupnorm folded into a Relu activation + speaker/gst layer bias + residuals),
key/query projections + L2 normalization, similarity bmm, CTC softmax /
log-softmax, sim_ctc bmm (one-hot gather as matmul), softplus log-sigmoid
terms, masked reductions, BOTH monotonic-alignment forward DPs (rows of the DP
computed with hardware tensor_tensor_scan in a negated min/add form) and the
backtrack decision bits D = (i==j) | (v[i-1,j-1] > v[i,j-1]).

Host: tiny exact backtrack walks over the D bits, log-space CTC scan, scalar
loss assembly, one-hot attention build.
"""
import numpy as np

import concourse.bass as bass
import concourse.bacc as bacc
import concourse.mybir as mybir
import concourse.tile as tile
from concourse.bass_utils import run_bass_kernel_spmd
from concourse.masks import make_identity

F32 = mybir.dt.float32
U8 = mybir.dt.uint8
I32 = mybir.dt.int32
AF = mybir.ActivationFunctionType
OP = mybir.AluOpType
AX = mybir.AxisListType

N, TX, TDEC = 16, 256, 1024
MEL = 80
ATT_H = 128
VOCAB = 100
NL = 6
C = 256
NCORES = 8
NB = 2
BIGPOS = 1e9
TP = TDEC + 2
VB = TDEC + 1  # vtab block width (guard col + 1024)


# ----------------------------------------------------------------------------
# device program
# ----------------------------------------------------------------------------

def build_nc(phase=None):
    import os
    phase = phase or os.environ.get("KPHASE", "all")
    P = ["conv", "sim", "ctc", "scan", "d", "all"].index(phase)
    nc = bacc.Bacc("TRN2")
    dt = nc.dram_tensor
    ins = dict(
        xm=dt("xm", [NB, MEL, TP], F32, kind="ExternalInput"),
        spec_y=dt("spec_y", [NB, 8, 128, MEL], F32, kind="ExternalInput"),
        textT=dt("textT", [NB, 257, TX], F32, kind="ExternalInput"),
        eWt=dt("eWt", [257, ATT_H], F32, kind="ExternalInput"),
        qWt=dt("qWt", [257, ATT_H - 1], F32, kind="ExternalInput"),
        cWt=dt("cWt", [257, VOCAB], F32, kind="ExternalInput"),
        w0t_m=dt("w0t_m", [3, MEL, C], F32, kind="ExternalInput"),
        w0t_a=dt("w0t_a", [3, MEL, C], F32, kind="ExternalInput"),
        wrt_m=dt("wrt_m", [NL - 1, 3, C, C], F32, kind="ExternalInput"),
        wrt_a=dt("wrt_a", [NL - 1, 3, C, C], F32, kind="ExternalInput"),
        gnw_m=dt("gnw_m", [NL, C], F32, kind="ExternalInput"),
        gnb_m=dt("gnb_m", [NL, C], F32, kind="ExternalInput"),
        gnw_a=dt("gnw_a", [NL, C], F32, kind="ExternalInput"),
        gnb_a=dt("gnb_a", [NL, C], F32, kind="ExternalInput"),
        lb_m=dt("lb_m", [NL, NB, C], F32, kind="ExternalInput"),
        lb_a=dt("lb_a", [NL, NB, C], F32, kind="ExternalInput"),
        smask=dt("smask", [NB, TP], F32, kind="ExternalInput"),
        tmask=dt("tmask", [NB, TX], F32, kind="ExternalInput"),
        ehotT=dt("ehotT", [NB, VOCAB, TX], F32, kind="ExternalInput"),
        consts=dt("consts", [1, 8], F32, kind="ExternalInput"),
        rowidx=dt("rowidx", [128, 8], F32, kind="ExternalInput"),
    )
    outs = dict(
        o_sim=dt("o_sim", [NB, 2, 128, TDEC], F32, kind="ExternalOutput"),
        o_lsm=dt("o_lsm", [NB, 8, 128, VOCAB], F32, kind="ExternalOutput"),
        o_db=dt("o_db", [128, 8 * TDEC], U8, kind="ExternalOutput"),
        o_mask=dt("o_mask", [NB, 2, 128, TDEC], F32, kind="ExternalOutput"),
        o_snls=dt("o_snls", [NB, 1], F32, kind="ExternalOutput"),
    )

    with tile.TileContext(nc) as tc:
        with (
            tc.tile_pool(name="per", bufs=1) as per,      # persistent, unique tags
            tc.tile_pool(name="xp", bufs=9) as xp,        # conv activations
            tc.tile_pool(name="wt", bufs=9) as wtp,      # conv weights
            tc.tile_pool(name="st", bufs=6) as stp,       # [128,1]-ish stats
            tc.tile_pool(name="rot", bufs=3) as rot,      # rotating [128,1024] work
            tc.tile_pool(name="cps", bufs=5, space="PSUM") as psp,   # [128,512]
            tc.tile_pool(name="sps", bufs=3, space="PSUM") as psp2,  # small psums
        ):
            ident = per.tile([128, 128], F32, tag="ident")
            make_identity(nc, ident)

            ones_col = per.tile([128, 1], F32, tag="ones_col")
            nc.vector.memset(ones_col, 1.0)
            ones_row = per.tile([1, 128], F32, tag="ones_row")
            nc.vector.memset(ones_row, 1.0)
            jio = per.tile([128, TDEC], F32, tag="jio")
            nc.gpsimd.iota(jio, pattern=[[1, TDEC]], base=0, channel_multiplier=0, allow_small_or_imprecise_dtypes=True)
            csts = per.tile([128, 8], F32, tag="csts")
            nc.gpsimd.dma_start(out=csts, in_=bass.AP(ins["consts"], 0, [[0, 128], [1, 8]]))
            sc10 = csts[:, 0:1]
            smb = csts[:, 1:2]
            epsc = csts[:, 2:3]
            rix = per.tile([128, 8], F32, tag="rix")
            nc.sync.dma_start(out=rix, in_=ins["rowidx"][:, :])

            gw_m = per.tile([128, NL * 2], F32, tag="gw_m")
            gb_m = per.tile([128, NL * 2], F32, tag="gb_m")
            gw_a = per.tile([128, NL * 2], F32, tag="gw_a")
            gb_a = per.tile([128, NL * 2], F32, tag="gb_a")
            lbm = per.tile([128, NL * NB * 2], F32, tag="lbm")
            lba = per.tile([128, NL * NB * 2], F32, tag="lba")
            # single strided DMA each: [128 part, (l, ct)] / [128, (l, b, ct)]
            for t_sb, t_dram in ((gw_m, "gnw_m"), (gb_m, "gnb_m"), (gw_a, "gnw_a"), (gb_a, "gnb_a")):
                nc.sync.dma_start(out=t_sb[:, :],
                                  in_=bass.AP(ins[t_dram], 0, [[1, 128], [C, NL], [128, 2]]))
            for t_sb, t_dram in ((lbm, "lb_m"), (lba, "lb_a")):
                nc.sync.dma_start(out=t_sb[:, :],
                                  in_=bass.AP(ins[t_dram], 0, [[1, 128], [NB * C, NL], [C, NB], [128, 2]]))

            maskbc = []
            for b in range(NB):
                mb = per.tile([128, TDEC], F32, tag=f"maskbc{b}")
                nc.gpsimd.dma_start(
                    out=mb, in_=bass.AP(ins["smask"], b * TP + 1, [[0, 128], [1, TDEC]]))
                maskbc.append(mb)
            tmcol = per.tile([128, NB * 2], F32, tag="tmcol")
            nc.sync.dma_start(out=tmcol[:, :],
                              in_=bass.AP(ins["tmask"], 0, [[1, 128], [TX, NB], [128, 2]]))

            d_val = nc.dram_tensor("d_val", [4, TX, TDEC], F32)
            d_vt0 = nc.dram_tensor("d_vt0", [4, 128, VB], F32)
            d_vt1 = nc.dram_tensor("d_vt1", [4, 128, VB], F32)
            NSLOT = 7
            VRING = 5
            slotring = per.tile([128, NSLOT * VB], F32, tag="slotring")
            valring = per.tile([128, VRING * TDEC], F32, tag="valring")
            simall = per.tile([128, 4 * TDEC], F32, tag="simall")
            m2all = per.tile([128, 4 * TDEC], F32, tag="m2all")
            lsmall = per.tile([128, 16 * VOCAB], F32, tag="lsmall")
            snall = per.tile([1, NB], F32, tag="snall")
            for s in range(NSLOT):
                nc.vector.memset(slotring[0:4, s * VB: s * VB + 1], BIGPOS)
            nc.vector.memset(slotring[0:4, (NSLOT - 1) * VB: (NSLOT - 1) * VB + TDEC], BIGPOS)


            def conv_stack(b, aux):
                w0 = ins["w0t_a" if aux else "w0t_m"]
                wr = ins["wrt_a" if aux else "wrt_m"]
                gw = gw_a if aux else gw_m
                gb = gb_a if aux else gb_m
                lb = lba if aux else lbm
                x0 = xp.tile([128, TP], F32, tag="x")
                nc.sync.dma_start(out=x0[:MEL, :], in_=ins["xm"][b])
                x = [x0]
                for l in range(NL):
                    cin_tiles = 1 if l == 0 else 2
                    kdim = MEL if l == 0 else 128
                    wts = []
                    for k in range(3):
                        for cit in range(cin_tiles):
                            w = wtp.tile([128, C], F32, tag="wt")
                            if l == 0:
                                nc.sync.dma_start(out=w[:MEL, :], in_=w0[k])
                            else:
                                nc.sync.dma_start(out=w, in_=wr[l - 1, k, cit * 128:(cit + 1) * 128, :])
                            wts.append(w)
                    xn = []
                    for ct in range(2):
                        stats = stp.tile([128, 2, 6], F32, tag="bnst")
                        pss = []
                        for lc in range(2):
                            ps = psp.tile([128, 512], F32, tag="cps")
                            mm = 0
                            for k in range(3):
                                for cit in range(cin_tiles):
                                    nc.tensor.matmul(
                                        ps,
                                        wts[k * cin_tiles + cit][:kdim, ct * 128:(ct + 1) * 128],
                                        x[cit][:kdim, lc * 512 + k: lc * 512 + k + 512],
                                        start=(mm == 0), stop=(mm == 3 * cin_tiles - 1))
                                    mm += 1
                            nc.vector.bn_stats(out=stats[:, lc, :], in_=ps[:, :])
                            pss.append(ps)
                        mv = stp.tile([128, 2], F32, tag="bnmv")
                        nc.vector.bn_aggr(out=mv, in_=stats)
                        sd = stp.tile([128, 1], F32, tag="sd")
                        nc.scalar.activation(out=sd, in_=mv[:, 1:2], func=AF.Sqrt, bias=epsc)
                        rs = stp.tile([128, 1], F32, tag="rs")
                        nc.vector.reciprocal(rs, sd)
                        ga = stp.tile([128, 1], F32, tag="ga")
                        nc.vector.tensor_mul(ga, rs, gw[:, l * 2 + ct: l * 2 + ct + 1])
                        gbb = stp.tile([128, 1], F32, tag="gbb")
                        nc.vector.tensor_mul(gbb, mv[:, 0:1], ga)
                        nc.vector.tensor_sub(gbb, gb[:, l * 2 + ct: l * 2 + ct + 1], gbb)
                        xt_ = xp.tile([128, TP], F32, tag="x")
                        for lc in range(2):
                            nc.scalar.activation(out=xt_[:, 1 + lc * 512: 1 + (lc + 1) * 512],
                                                 in_=pss[lc], func=AF.Relu, scale=ga, bias=gbb)
                        lcol = lb[:, (l * NB + b) * 2 + ct: (l * NB + b) * 2 + ct + 1]
                        if l % 2 == 1:
                            # (lbias + y) + x_prev, then mask
                            nc.vector.scalar_tensor_tensor(
                                out=xt_[:, 1:1 + TDEC], in0=xt_[:, 1:1 + TDEC], scalar=lcol,
                                in1=x[ct][:, 1:1 + TDEC], op0=OP.add, op1=OP.add)
                            if l < NL - 1:
                                nc.vector.tensor_mul(xt_[:, 1:1 + TDEC], xt_[:, 1:1 + TDEC],
                                                     maskbc[b])
                        else:
                            # (lbias + y) * mask in one op
                            if l < NL - 1:
                                nc.vector.scalar_tensor_tensor(
                                    out=xt_[:, 1:1 + TDEC], in0=xt_[:, 1:1 + TDEC], scalar=lcol,
                                    in1=maskbc[b], op0=OP.add, op1=OP.mult)
                            else:
                                nc.vector.tensor_scalar_add(xt_[:, 1:1 + TDEC],
                                                            xt_[:, 1:1 + TDEC], lcol)
                        nc.vector.memset(xt_[:, 0:1], 0.0)
                        nc.vector.memset(xt_[:, TP - 1: TP], 0.0)
                        xn.append(xt_)
                    x = xn
                return x

            def normalize_rows(t, n_free):
                scr = rot.tile([128, n_free], F32, tag="sw")
                sq = stp.tile([128, 1], F32, tag="nsq")
                nc.vector.tensor_mul(scr, t[:, :n_free], t[:, :n_free])
                nc.vector.reduce_sum(sq, scr, axis=AX.X)
                sd = stp.tile([128, 1], F32, tag="nsd")
                nc.scalar.activation(out=sd, in_=sq, func=AF.Sqrt)
                rr = stp.tile([128, 1], F32, tag="nrr")
                nc.vector.reciprocal(rr, sd)
                nc.vector.tensor_scalar_mul(t[:, :n_free], t[:, :n_free], rr)

            def stage_val(val, b, m, xt):
                p = 2 * m + b
                nc.sync.dma_start(out=d_val[p, xt * 128:(xt + 1) * 128, :], in_=val)

            # ---------------- main stacks, key/query, similarity ----------------
            m2_tiles = {}
            x6a_all = {}
            for b in range(NB):
                x6 = conv_stack(b, aux=False)
                if P >= 2:
                    x6a_all[b] = conv_stack(b, aux=True)
                if P < 1:
                    continue
                SUB = int(os.environ.get("KSUB", "9"))

                # key
                tta = [per.tile([128, TX], F32, tag=f"tt{i}", name=f"tt{i}") for i in range(2)]
                ttb = per.tile([1, TX], F32, tag="ttb")
                for i in range(2):
                    nc.sync.dma_start(out=tta[i], in_=ins["textT"][b, i * 128:(i + 1) * 128, :])
                nc.sync.dma_start(out=ttb, in_=ins["textT"][b, 256:257, :])
                if b == 0:
                    eW = [per.tile([128, ATT_H], F32, tag=f"eW{i}", name=f"eW{i}") for i in range(2)]
                    eWb = per.tile([1, ATT_H], F32, tag="eWb")
                    for i in range(2):
                        nc.sync.dma_start(out=eW[i], in_=ins["eWt"][i * 128:(i + 1) * 128, :])
                    nc.sync.dma_start(out=eWb, in_=ins["eWt"][256:257, :])
                    qW = [per.tile([128, ATT_H - 1], F32, tag=f"qW{i}", name=f"qW{i}") for i in range(2)]
                    qWb = per.tile([1, ATT_H - 1], F32, tag="qWb")
                    for i in range(2):
                        nc.sync.dma_start(out=qW[i], in_=ins["qWt"][i * 128:(i + 1) * 128, :])
                    nc.sync.dma_start(out=qWb, in_=ins["qWt"][256:257, :])
                keyT = per.tile([128, TX], F32, tag=f"keyT{b}")
                nc.vector.memset(keyT, 0.0)
                for xt in range(2):
                    if SUB < 1:
                        continue
                    pk = psp2.tile([128, ATT_H], F32, tag="sp")
                    for i in range(2):
                        nc.tensor.matmul(pk, tta[i][:, xt * 128:(xt + 1) * 128], eW[i],
                                         start=(i == 0), stop=False)
                    nc.tensor.matmul(pk, ttb[:, xt * 128:(xt + 1) * 128], eWb,
                                     start=False, stop=True)
                    kk = rot.tile([128, ATT_H], F32, tag="sw")
                    nc.scalar.activation(out=kk, in_=pk, func=AF.Copy)
                    if SUB >= 2:
                        normalize_rows(kk, ATT_H)
                    if SUB >= 3:
                        pt = psp2.tile([128, 128], F32, tag="sp")
                        nc.tensor.transpose(pt, kk, ident)
                        nc.scalar.activation(out=keyT[:, xt * 128:(xt + 1) * 128], in_=pt, func=AF.Copy)

                if SUB < 4:
                    continue
                # query
                qT = per.tile([128, TDEC], F32, tag="qT")
                for yt in range(8):
                    pq = psp2.tile([128, ATT_H - 1], F32, tag="sp")
                    for i in range(2):
                        nc.tensor.matmul(pq, x6[i][:, 1 + yt * 128: 1 + (yt + 1) * 128], qW[i],
                                         start=(i == 0), stop=False)
                    nc.tensor.matmul(pq, ones_row, qWb,
                                     start=False, stop=True)
                    qq = rot.tile([128, 128], F32, tag="sw")
                    nc.scalar.activation(out=qq[:, 0:ATT_H - 1], in_=pq, func=AF.Copy)
                    sy = rot.tile([128, MEL], F32, tag="sw")
                    nc.sync.dma_start(out=sy, in_=ins["spec_y"][b, yt])
                    en = stp.tile([128, 1], F32, tag="en")
                    nc.vector.reduce_sum(en, sy, axis=AX.X)
                    nc.vector.tensor_scalar_mul(qq[:, ATT_H - 1: ATT_H], en, 1.0 / MEL)
                    normalize_rows(qq, ATT_H)
                    pt = psp2.tile([128, 128], F32, tag="sp")
                    nc.tensor.transpose(pt, qq, ident)
                    nc.scalar.activation(out=qT[:, yt * 128:(yt + 1) * 128], in_=pt, func=AF.Copy)

                if SUB < 5:
                    continue
                # similarity + masked softplus sums + val1
                snps = []
                for xt in range(2):
                    bx = b * 2 + xt
                    ssim = simall[:, bx * TDEC:(bx + 1) * TDEC]
                    for yc in range(2):
                        psim = psp.tile([128, 512], F32, tag="cps")
                        nc.tensor.matmul(psim, keyT[:, xt * 128:(xt + 1) * 128],
                                         qT[:, yc * 512:(yc + 1) * 512], start=True, stop=True)
                        nc.scalar.activation(out=ssim[:, yc * 512:(yc + 1) * 512], in_=psim,
                                             func=AF.Identity, scale=sc10, bias=smb)
                    m2 = m2all[:, bx * TDEC:(bx + 1) * TDEC]
                    nc.vector.tensor_scalar_mul(m2, maskbc[b], tmcol[:, b * 2 + xt: b * 2 + xt + 1])
                    m2_tiles[(b, xt)] = m2
                    # nlsn = softplus(sim) = relu(sim) + ln(1 + exp(-|sim|))
                    ab = rot.tile([128, TDEC], F32, tag="work")
                    nc.scalar.activation(out=ab, in_=ssim, func=AF.Abs)
                    nc.scalar.activation(out=ab, in_=ab, func=AF.Exp, scale=-1.0)
                    nc.scalar.activation(out=ab, in_=ab, func=AF.Ln, bias=ones_col)
                    nlsn = rot.tile([128, TDEC], F32, tag="work")
                    nc.scalar.activation(out=nlsn, in_=ssim, func=AF.Relu)
                    nc.vector.tensor_add(nlsn, nlsn, ab)
                    scr = rot.tile([128, TDEC], F32, tag="work")
                    snp = stp.tile([128, 1], F32, tag=f"snp{xt}")
                    nc.vector.tensor_mul(scr, nlsn, m2)
                    nc.vector.reduce_sum(snp, scr, axis=AX.X)
                    snps.append(snp)
                    # softplus(-sim) = softplus(sim) - sim
                    lsn = rot.tile([128, TDEC], F32, tag="work")
                    nc.vector.tensor_sub(lsn, nlsn, ssim)
                    val = rot.tile([128, TDEC], F32, tag="work")
                    nc.vector.tensor_mul(val, lsn, m2)
                    stage_val(val, b, 0, xt)
                sn2 = stp.tile([128, 1], F32, tag="sn2")
                nc.vector.tensor_add(sn2, snps[0], snps[1])
                psn = psp2.tile([1, 1], F32, tag="sp")
                nc.tensor.matmul(psn, sn2, ones_col, start=True, stop=True)
                nc.scalar.activation(out=snall[:, b:b + 1], in_=psn, func=AF.Copy)

            # ---------------- aux stacks, ctc, sim_ctc, val2 ----------------
            for b in range(NB if P >= 2 else 0):
                x6a = x6a_all[b]
                if b == 0:
                    cW = [per.tile([128, VOCAB], F32, tag=f"cW{i}", name=f"cW{i}") for i in range(2)]
                    cWb = per.tile([1, VOCAB], F32, tag="cWb")
                    for i in range(2):
                        nc.sync.dma_start(out=cW[i], in_=ins["cWt"][i * 128:(i + 1) * 128, :])
                    nc.sync.dma_start(out=cWb, in_=ins["cWt"][256:257, :])
                qTc = per.tile([128, TDEC], F32, tag="qTc")
                for yt in range(8):
                    pc = psp2.tile([128, VOCAB], F32, tag="sp")
                    for i in range(2):
                        nc.tensor.matmul(pc, x6a[i][:, 1 + yt * 128: 1 + (yt + 1) * 128], cW[i],
                                         start=(i == 0), stop=False)
                    nc.tensor.matmul(pc, ones_row, cWb,
                                     start=False, stop=True)
                    mx = stp.tile([128, 1], F32, tag="mx")
                    nc.vector.reduce_max(mx, pc, axis=AX.X)
                    nmx = stp.tile([128, 1], F32, tag="nmx")
                    nc.vector.tensor_scalar_mul(nmx, mx, -1.0)
                    exb = rot.tile([128, VOCAB], F32, tag="sw")
                    nc.scalar.activation(out=exb, in_=pc, func=AF.Exp, bias=nmx)
                    s = stp.tile([128, 1], F32, tag="s")
                    nc.vector.reduce_sum(s, exb, axis=AX.X)
                    lns = stp.tile([128, 1], F32, tag="lns")
                    nc.scalar.activation(out=lns, in_=s, func=AF.Ln)
                    r = stp.tile([128, 1], F32, tag="r")
                    nc.vector.reciprocal(r, s)
                    qsb = rot.tile([128, VOCAB], F32, tag="sw")
                    nc.vector.tensor_scalar_mul(qsb, exb, r)
                    nb_ = stp.tile([128, 1], F32, tag="nb_")
                    nc.vector.tensor_add(nb_, mx, lns)
                    nc.vector.tensor_scalar_mul(nb_, nb_, -1.0)
                    lidx = b * 8 + yt
                    nc.scalar.activation(out=lsmall[:, lidx * VOCAB:(lidx + 1) * VOCAB],
                                         in_=pc, func=AF.Identity, bias=nb_)
                    ptc = psp2.tile([128, 128], F32, tag="sp")
                    nc.tensor.transpose(ptc[:VOCAB, :], qsb, ident)
                    nc.scalar.activation(out=qTc[:VOCAB, yt * 128:(yt + 1) * 128],
                                         in_=ptc[:VOCAB, :], func=AF.Copy)
                eh = per.tile([128, TX], F32, tag="eh")
                nc.sync.dma_start(out=eh[:VOCAB, :], in_=ins["ehotT"][b])
                for xt in range(2):
                    scs = rot.tile([128, TDEC], F32, tag="work")
                    for yc in range(2):
                        psc = psp.tile([128, 512], F32, tag="cps")
                        nc.tensor.matmul(psc, eh[:VOCAB, xt * 128:(xt + 1) * 128],
                                         qTc[:VOCAB, yc * 512:(yc + 1) * 512],
                                         start=True, stop=True)
                        nc.scalar.activation(out=scs[:, yc * 512:(yc + 1) * 512], in_=psc,
                                             func=AF.Copy)
                    # softplus(-simc) = relu(-simc) + ln(1 + exp(-|simc|))
                    ab2 = rot.tile([128, TDEC], F32, tag="work")
                    nc.scalar.activation(out=ab2, in_=scs, func=AF.Abs)
                    nc.scalar.activation(out=ab2, in_=ab2, func=AF.Exp, scale=-1.0)
                    nc.scalar.activation(out=ab2, in_=ab2, func=AF.Ln, bias=ones_col)
                    lsn2 = rot.tile([128, TDEC], F32, tag="work")
                    nc.scalar.activation(out=lsn2, in_=scs, func=AF.Relu, scale=-1.0)
                    nc.vector.tensor_add(lsn2, lsn2, ab2)
                    val2 = rot.tile([128, TDEC], F32, tag="work")
                    nc.vector.tensor_mul(val2, lsn2, m2_tiles[(b, xt)])
                    stage_val(val2, b, 1, xt)

            # ---------------- MAS forward scans ----------------

            # rows beyond 223 can never be touched: text_lengths < 225 and the DP
            # flows strictly downward in i, so cap the scan row count.
            TX_SCAN = 224
            for i in range(TX_SCAN if P >= 3 else 0):
                s = i % NSLOT
                sv = i % VRING
                nc.sync.dma_start(out=valring[0:4, sv * TDEC:(sv + 1) * TDEC],
                                  in_=d_val[0:4, i, :])
                if i == 0:
                    data0 = slotring[0:4, (NSLOT - 1) * VB: (NSLOT - 1) * VB + TDEC]
                    init = 0.0
                else:
                    sp_ = (i - 1) % NSLOT
                    data0 = slotring[0:4, sp_ * VB: sp_ * VB + TDEC]
                    init = BIGPOS
                nc.vector.tensor_tensor_scan(
                    out=slotring[0:4, s * VB + 1: s * VB + 1 + TDEC],
                    data0=data0,
                    data1=valring[0:4, sv * TDEC:(sv + 1) * TDEC],
                    initial=init, op0=OP.min, op1=OP.add)
                dvt_out = d_vt0[0:4, i, :] if i < 128 else d_vt1[0:4, i - 128, :]
                nc.sync.dma_start(out=dvt_out, in_=slotring[0:4, s * VB: (s + 1) * VB])

            # ---------------- D bits ----------------
            dbu = per.tile([128, 8 * TDEC], U8, tag="dbu")
            for g in range(2 if P >= 4 else 0):
                band = rot.tile([128, TDEC], F32, tag="work", name=f"band{g}")
                nc.vector.tensor_scalar(band, jio, rix[:, g:g + 1], None, op0=OP.is_equal)
                for p in range(4):
                    Ab = rot.tile([128, VB], F32, tag="ab", name=f"Ab{g}{p}")
                    Bb = rot.tile([128, VB], F32, tag="ab", name=f"Bb{g}{p}")
                    if g == 0:
                        nc.sync.dma_start(out=Ab, in_=d_vt0[p, :, :])
                        nc.vector.memset(Bb[0:32, :], 0.0)
                        nc.sync.dma_start(out=Bb[1:128, :], in_=d_vt0[p, 0:127, :])
                    else:
                        nc.vector.memset(Ab[96:128, :], 0.0)
                        nc.sync.dma_start(out=Ab[0:96, :], in_=d_vt1[p, 0:96, :])
                        nc.vector.memset(Bb[96:128, :], 0.0)
                        nc.sync.dma_start(out=Bb[0:1, :], in_=d_vt0[p, 127:128, :])
                        nc.sync.dma_start(out=Bb[1:96, :], in_=d_vt1[p, 0:95, :])
                    cmpf = rot.tile([128, TDEC], F32, tag="work", name=f"cmpf{g}{p}")
                    nc.vector.tensor_tensor(out=cmpf, in0=Bb[:, 0:TDEC], in1=Ab[:, 0:TDEC],
                                            op=OP.is_lt)
                    nc.vector.tensor_tensor(out=dbu[:, (g * 4 + p) * TDEC:(g * 4 + p + 1) * TDEC],
                                            in0=cmpf, in1=band, op=OP.max)
            nc.sync.dma_start(out=outs["o_db"][:, :], in_=dbu)
            # out APs iterate (q, c, inner) to match the sbuf staging layout
            sim_ap = bass.AP(outs["o_sim"], 0, [[TDEC, 128], [2 * 128 * TDEC, NB], [128 * TDEC, 2], [1, TDEC]])
            nc.sync.dma_start(out=sim_ap, in_=simall.rearrange("p (c t) -> p c t", c=4))
            mask_ap = bass.AP(outs["o_mask"], 0, [[TDEC, 128], [2 * 128 * TDEC, NB], [128 * TDEC, 2], [1, TDEC]])
            nc.sync.dma_start(out=mask_ap, in_=m2all.rearrange("p (c t) -> p c t", c=4))
            lsm_ap = bass.AP(outs["o_lsm"], 0, [[VOCAB, 128], [8 * 128 * VOCAB, NB], [128 * VOCAB, 8], [1, VOCAB]])
            nc.sync.dma_start(out=lsm_ap, in_=lsmall.rearrange("p (c v) -> p c v", c=16))
            nc.sync.dma_start(out=outs["o_snls"][:, :], in_=snall)

    nc.finalize()
    return nc


# ----------------------------------------------------------------------------
# host: input prep
# ----------------------------------------------------------------------------

def _relu(x):
    return np.maximum(x, 0.0)


def _mlp2(v, w1, b1, w2, b2):
    h = _relu(v @ w1.T + b1)
    return _relu(h @ w2.T + b2)


def _prep_shared(params):
    p = {}
    aug = lambda w, b: np.ascontiguousarray(
        np.concatenate([np.asarray(w, np.float32).T, np.asarray(b, np.float32)[None, :]], 0))
    p["eWt"] = aug(params["enc_proj_w"], params["enc_proj_b"])
    p["qWt"] = aug(params["query_proj_w"], params["query_proj_b"])
    p["cWt"] = aug(params["ctc_proj_w"], params["ctc_proj_b"])
    for stk, tag in ((params["main"], "m"), (params["aux"], "a")):
        p[f"w0t_{tag}"] = np.ascontiguousarray(
            np.asarray(stk[0]["conv_w"], np.float32).transpose(2, 1, 0))
        p[f"wrt_{tag}"] = np.ascontiguousarray(np.stack(
            [np.asarray(stk[l]["conv_w"], np.float32).transpose(2, 1, 0) for l in range(1, NL)]))
        p[f"gnw_{tag}"] = np.ascontiguousarray(
            np.stack([np.asarray(stk[l]["gn_w"], np.float32) for l in range(NL)]))
        p[f"gnb_{tag}"] = np.ascontiguousarray(
            np.stack([np.asarray(stk[l]["gn_b"], np.float32) for l in range(NL)]))
    p["consts"] = np.array(
        [[10.0 * np.exp(np.float32(params["sim_w"])), np.float32(params["sim_b"]),
          1e-5, 0, 0, 0, 0, 0]], np.float32)
    ridx = np.empty((128, 8), np.float32)
    for pp in range(128):
        for g in range(8):
            ridx[pp, g] = 128 * g + pp
    p["rowidx"] = ridx
    return p


def _prep_core(c, text, spec, spkr_vec, gst_vec, text_lengths, spec_lengths,
               enc_input, params, shared):
    sl = slice(2 * c, 2 * c + 2)
    spec_c = np.asarray(spec[sl], np.float32)
    tl = np.asarray(text_lengths[sl]).astype(np.int64)
    sll = np.asarray(spec_lengths[sl]).astype(np.int64)
    smask = (np.arange(TDEC)[None, :] < sll[:, None]).astype(np.float32)
    d = dict(shared)
    xm = np.zeros((NB, MEL, TP), np.float32)
    xm[:, :, 1:1 + TDEC] = (spec_c * smask[:, :, None]).transpose(0, 2, 1)
    d["xm"] = xm
    d["spec_y"] = np.ascontiguousarray(spec_c.reshape(NB, 8, 128, MEL))
    tt = np.empty((NB, 257, TX), np.float32)
    tt[:, :256] = np.asarray(text[sl], np.float32).transpose(0, 2, 1)
    tt[:, 256] = 1.0
    d["textT"] = tt
    sm = np.zeros((NB, TP), np.float32)
    sm[:, 1:1 + TDEC] = smask
    d["smask"] = sm
    d["tmask"] = (np.arange(TX)[None, :] < tl[:, None]).astype(np.float32)
    eh = np.zeros((NB, VOCAB, TX), np.float32)
    ei = np.asarray(enc_input[sl]).astype(np.int64)
    for b in range(NB):
        eh[b, ei[b], np.arange(TX)] = 1.0
    d["ehotT"] = eh
    sv = np.asarray(spkr_vec[sl], np.float32)
    gv = np.asarray(gst_vec[sl], np.float32)
    for stk, tag in ((params["main"], "m"), (params["aux"], "a")):
        lb = np.empty((NL, NB, C), np.float32)
        for l in range(NL):
            pl = stk[l]
            lb[l] = (_mlp2(sv, np.asarray(pl["s1w"], np.float32), np.asarray(pl["s1b"], np.float32),
                           np.asarray(pl["s2w"], np.float32), np.asarray(pl["s2b"], np.float32))
                     + _mlp2(gv, np.asarray(pl["g1w"], np.float32), np.asarray(pl["g1b"], np.float32),
                             np.asarray(pl["g2w"], np.float32), np.asarray(pl["g2b"], np.float32)))
        d[f"lb_{tag}"] = lb
    return d


# ----------------------------------------------------------------------------
# host: post-processing
# ----------------------------------------------------------------------------

def _backtrack(D, t_x, t_y):
    """D [B,TX,TDEC] uint8; returns idx trajectories [B, TDEC] int64."""
    B = D.shape[0]
    bi = np.arange(B)
    index = (t_x - 1).astype(np.int64).copy()
    idx_traj = np.empty((B, TDEC), np.int64)
    for j in range(TDEC - 1, -1, -1):
        idx_traj[:, j] = index
        write = j < t_y
        move = (index != 0) & (D[bi, index, j] != 0)
        index = np.where(write & move, index - 1, index)
    return idx_traj


def _ctc_loss(log_probs_nt, targets, input_lengths, target_lengths, blank=0):
    """log_probs_nt [N, T, V] f32. Reference-faithful log-space CTC."""
    Nb, T, V = log_probs_nt.shape
    S = targets.shape[1]
    L = 2 * S + 1
    NEG = -1e9
    ext = np.full((Nb, L), blank, np.int64)
    ext[:, 1::2] = targets
    skip = np.concatenate([np.zeros((Nb, 2), bool),
                           (ext[:, 2:] != blank) & (ext[:, 2:] != ext[:, :-2])], axis=1)
    lp_ext = np.take_along_axis(log_probs_nt, np.broadcast_to(ext[:, None, :], (Nb, T, L)), axis=2)
    lp_ext = np.ascontiguousarray(lp_ext.transpose(1, 0, 2), dtype=np.float32)  # [T,N,L]
    alpha = np.full((Nb, L), NEG, np.float32)
    alpha[:, 0] = lp_ext[0, :, 0]
    alpha[:, 1] = lp_ext[0, :, 1]
    a2 = np.empty_like(alpha)
    a3 = np.empty_like(alpha)
    for t in range(1, T):
        a2[:, 0] = NEG
        a2[:, 1:] = alpha[:, :-1]
        a3[:, :2] = NEG
        a3[:, 2:] = np.where(skip[:, 2:], alpha[:, :-2], NEG)
        new = (np.logaddexp(np.logaddexp(alpha, a2), a3) + lp_ext[t]).astype(np.float32)
        upd = t < input_lengths
        alpha[upd] = new[upd]
    bi = np.arange(Nb)
    e1 = alpha[bi, 2 * target_lengths]
    e2 = alpha[bi, 2 * target_lengths - 1]
    loss = -np.logaddexp(e1, e2)
    return np.float32(np.mean(loss / target_lengths.astype(np.float32)))


_NC_CACHE = None


def kernel(text, spec, text_lengths, spec_lengths, spkr_vec, gst_vec, enc_input, params):
    global _NC_CACHE
    text = np.asarray(text)
    spec = np.asarray(spec)
    text_lengths = np.asarray(text_lengths)
    spec_lengths = np.asarray(spec_lengths)
    enc_input = np.asarray(enc_input)

    if _NC_CACHE is None:
        _NC_CACHE = build_nc()
    nc = _NC_CACHE

    shared = _prep_shared(params)
    in_maps = [
        _prep_core(c, text, spec, spkr_vec, gst_vec, text_lengths, spec_lengths,
                   enc_input, params, shared)
        for c in range(NCORES)
    ]
    res = run_bass_kernel_spmd(nc, in_maps, core_ids=list(range(NCORES)))
    results = res.results

    tl = text_lengths.astype(np.int64)
    sl = spec_lengths.astype(np.int64)
    sim = np.empty((N, TX, TDEC), np.float32)
    att_mask = np.empty((N, TX, TDEC), np.float32)
    lsm = np.empty((N, TDEC, VOCAB), np.float32)
    snls = np.empty((N,), np.float32)
    D1 = np.empty((N, TX, TDEC), np.uint8)
    D2 = np.empty((N, TX, TDEC), np.uint8)
    for c in range(NCORES):
        r = results[c]
        s2 = slice(2 * c, 2 * c + 2)
        sim[s2] = r["o_sim"].reshape(NB, TX, TDEC)
        att_mask[s2] = r["o_mask"].reshape(NB, TX, TDEC)
        lsm[s2] = r["o_lsm"].reshape(NB, TDEC, VOCAB)
        snls[s2] = r["o_snls"][:, 0]
        Db = r["o_db"].reshape(128, 2, 4, TDEC).transpose(2, 1, 0, 3).reshape(4, TX, TDEC)
        D1[2 * c] = Db[0]
        D1[2 * c + 1] = Db[1]
        D2[2 * c] = Db[2]
        D2[2 * c + 1] = Db[3]

    idx1 = _backtrack(D1, tl, sl)
    idx2 = _backtrack(D2, tl, sl)

    bi = np.arange(N)[:, None]
    jj = np.arange(TDEC)[None, :]
    wmask = jj < sl[:, None]

    attention = np.zeros((N, TX, TDEC), np.float32)
    bidx, jidx = np.nonzero(wmask)
    attention[bidx, idx1[bidx, jidx], jidx] = 1.0

    # losses
    denom = (tl * sl).astype(np.float32)
    path_sim = np.sum(sim[bi, idx1, jj] * wmask, axis=1, dtype=np.float64).astype(np.float32)
    icl = (snls - path_sim) / denom
    nll = np.float32(np.mean(icl))

    aux_sim = np.zeros((N,), np.float64)
    for dshift in (-1, 0, 1):
        ii = idx2 + dshift
        ok = wmask & (ii >= 0) & (ii < tl[:, None])
        aux_sim += np.sum(sim[bi, np.clip(ii, 0, TX - 1), jj] * ok, axis=1, dtype=np.float64)
    aux_l = (snls - aux_sim.astype(np.float32)) / denom * 0.5

    ctc = _ctc_loss(lsm, enc_input.astype(np.int64), sl, tl)
    att_loss = np.float32(nll + np.float32(np.mean(aux_l)) + ctc)

    return attention, att_loss, att_mask, np.float32(nll)


# revision 34
# speedup vs baseline: 1.1348x; 1.0363x over previous
"""Trainium2 Bass kernel for nn_Attention_40767829574453 (Glow-TTS style aligner).

Sharding: pure data parallelism, batch 16 -> 8 cores x 2 batches each.

Device (per core): both 6-layer conv stacks (masked conv1d as shifted matmuls +
groupnorm folded into a Relu activation + speaker/gst layer bias + residuals),
key/query projections + L2 normalization, similarity bmm, CTC softmax /
log-softmax, sim_ctc bmm (one-hot gather as matmul), softplus log-sigmoid
terms, masked reductions, BOTH monotonic-alignment forward DPs (rows of the DP
computed with hardware tensor_tensor_scan in a negated min/add form) and the
backtrack decision bits D = (i==j) | (v[i-1,j-1] > v[i,j-1]).

Host: tiny exact backtrack walks over the D bits, log-space CTC scan, scalar
loss assembly, one-hot attention build.
"""
import numpy as np

import concourse.bass as bass
import concourse.bacc as bacc
import concourse.mybir as mybir
import concourse.tile as tile
from concourse.bass_utils import run_bass_kernel_spmd
from concourse.masks import make_identity

F32 = mybir.dt.float32
U8 = mybir.dt.uint8
I32 = mybir.dt.int32
AF = mybir.ActivationFunctionType
OP = mybir.AluOpType
AX = mybir.AxisListType

N, TX, TDEC = 16, 256, 1024
MEL = 80
ATT_H = 128
VOCAB = 100
NL = 6
C = 256
NCORES = 8
NB = 2
BIGPOS = 1e9
TP = TDEC + 2
VB = TDEC + 1  # vtab block width (guard col + 1024)


# ----------------------------------------------------------------------------
# device program
# ----------------------------------------------------------------------------

def build_nc(phase=None):
    import os
    phase = phase or os.environ.get("KPHASE", "all")
    P = ["conv", "sim", "ctc", "scan", "d", "all"].index(phase)
    nc = bacc.Bacc("TRN2")
    dt = nc.dram_tensor
    ins = dict(
        xm=dt("xm", [NB, MEL, TP], F32, kind="ExternalInput"),
        spec_y=dt("spec_y", [NB, 8, 128, MEL], F32, kind="ExternalInput"),
        textT=dt("textT", [NB, 257, TX], F32, kind="ExternalInput"),
        eWt=dt("eWt", [257, ATT_H], F32, kind="ExternalInput"),
        qWt=dt("qWt", [257, ATT_H - 1], F32, kind="ExternalInput"),
        cWt=dt("cWt", [257, VOCAB], F32, kind="ExternalInput"),
        w0t_m=dt("w0t_m", [3, MEL, C], F32, kind="ExternalInput"),
        w0t_a=dt("w0t_a", [3, MEL, C], F32, kind="ExternalInput"),
        wrt_m=dt("wrt_m", [NL - 1, 3, C, C], F32, kind="ExternalInput"),
        wrt_a=dt("wrt_a", [NL - 1, 3, C, C], F32, kind="ExternalInput"),
        gnw_m=dt("gnw_m", [NL, C], F32, kind="ExternalInput"),
        gnb_m=dt("gnb_m", [NL, C], F32, kind="ExternalInput"),
        gnw_a=dt("gnw_a", [NL, C], F32, kind="ExternalInput"),
        gnb_a=dt("gnb_a", [NL, C], F32, kind="ExternalInput"),
        lb_m=dt("lb_m", [NL, NB, C], F32, kind="ExternalInput"),
        lb_a=dt("lb_a", [NL, NB, C], F32, kind="ExternalInput"),
        smask=dt("smask", [NB, TP], F32, kind="ExternalInput"),
        tmask=dt("tmask", [NB, TX], F32, kind="ExternalInput"),
        ehotT=dt("ehotT", [NB, VOCAB, TX], F32, kind="ExternalInput"),
        consts=dt("consts", [1, 8], F32, kind="ExternalInput"),
        rowidx=dt("rowidx", [128, 8], F32, kind="ExternalInput"),
    )
    outs = dict(
        o_sim=dt("o_sim", [NB, 2, 128, TDEC], F32, kind="ExternalOutput"),
        o_lsm=dt("o_lsm", [NB, 8, 128, VOCAB], F32, kind="ExternalOutput"),
        o_db=dt("o_db", [128, 8 * TDEC], U8, kind="ExternalOutput"),
        o_mask=dt("o_mask", [NB, 2, 128, TDEC], F32, kind="ExternalOutput"),
        o_snls=dt("o_snls", [NB, 1], F32, kind="ExternalOutput"),
    )

    with tile.TileContext(nc) as tc:
        with (
            tc.tile_pool(name="per", bufs=1) as per,      # persistent, unique tags
            tc.tile_pool(name="xp", bufs=9) as xp,        # conv activations
            tc.tile_pool(name="wt", bufs=9) as wtp,      # conv weights
            tc.tile_pool(name="st", bufs=6) as stp,       # [128,1]-ish stats
            tc.tile_pool(name="rot", bufs=3) as rot,      # rotating [128,1024] work
            tc.tile_pool(name="cps", bufs=5, space="PSUM") as psp,   # [128,512]
            tc.tile_pool(name="sps", bufs=3, space="PSUM") as psp2,  # small psums
        ):
            ident = per.tile([128, 128], F32, tag="ident")
            make_identity(nc, ident)

            ones_col = per.tile([128, 1], F32, tag="ones_col")
            nc.vector.memset(ones_col, 1.0)
            ones_row = per.tile([1, 128], F32, tag="ones_row")
            nc.vector.memset(ones_row, 1.0)
            jio = per.tile([128, TDEC], F32, tag="jio")
            nc.gpsimd.iota(jio, pattern=[[1, TDEC]], base=0, channel_multiplier=0, allow_small_or_imprecise_dtypes=True)
            csts = per.tile([128, 8], F32, tag="csts")
            nc.gpsimd.dma_start(out=csts, in_=bass.AP(ins["consts"], 0, [[0, 128], [1, 8]]))
            sc10 = csts[:, 0:1]
            smb = csts[:, 1:2]
            epsc = csts[:, 2:3]
            rix = per.tile([128, 8], F32, tag="rix")
            nc.sync.dma_start(out=rix, in_=ins["rowidx"][:, :])

            gw_m = per.tile([128, NL * 2], F32, tag="gw_m")
            gb_m = per.tile([128, NL * 2], F32, tag="gb_m")
            gw_a = per.tile([128, NL * 2], F32, tag="gw_a")
            gb_a = per.tile([128, NL * 2], F32, tag="gb_a")
            lbm = per.tile([128, NL * NB * 2], F32, tag="lbm")
            lba = per.tile([128, NL * NB * 2], F32, tag="lba")
            # single strided DMA each: [128 part, (l, ct)] / [128, (l, b, ct)]
            for t_sb, t_dram in ((gw_m, "gnw_m"), (gb_m, "gnb_m"), (gw_a, "gnw_a"), (gb_a, "gnb_a")):
                nc.sync.dma_start(out=t_sb[:, :],
                                  in_=bass.AP(ins[t_dram], 0, [[1, 128], [C, NL], [128, 2]]))
            for t_sb, t_dram in ((lbm, "lb_m"), (lba, "lb_a")):
                nc.sync.dma_start(out=t_sb[:, :],
                                  in_=bass.AP(ins[t_dram], 0, [[1, 128], [NB * C, NL], [C, NB], [128, 2]]))

            maskbc = []
            for b in range(NB):
                mb = per.tile([128, TDEC], F32, tag=f"maskbc{b}")
                nc.gpsimd.dma_start(
                    out=mb, in_=bass.AP(ins["smask"], b * TP + 1, [[0, 128], [1, TDEC]]))
                maskbc.append(mb)
            tmcol = per.tile([128, NB * 2], F32, tag="tmcol")
            nc.sync.dma_start(out=tmcol[:, :],
                              in_=bass.AP(ins["tmask"], 0, [[1, 128], [TX, NB], [128, 2]]))

            d_val = nc.dram_tensor("d_val", [4, TX, TDEC], F32)
            dvt = [[nc.dram_tensor(f"d_vt{h}{par}", [4, 64, VB], F32)
                    for par in range(2)] for h in range(2)]
            NSLOT = 7
            VRING = 5
            slotring = per.tile([128, NSLOT * VB], F32, tag="slotring")
            valring = per.tile([128, VRING * TDEC], F32, tag="valring")
            simall = per.tile([128, 4 * TDEC], F32, tag="simall")
            m2all = per.tile([128, 4 * TDEC], F32, tag="m2all")
            lsmall = per.tile([128, 16 * VOCAB], F32, tag="lsmall")
            snall = per.tile([1, NB], F32, tag="snall")
            for s in range(NSLOT):
                nc.vector.memset(slotring[0:4, s * VB: s * VB + 1], BIGPOS)
            nc.vector.memset(slotring[0:4, (NSLOT - 1) * VB: (NSLOT - 1) * VB + TDEC], BIGPOS)


            def conv_stack(b, aux):
                w0 = ins["w0t_a" if aux else "w0t_m"]
                wr = ins["wrt_a" if aux else "wrt_m"]
                gw = gw_a if aux else gw_m
                gb = gb_a if aux else gb_m
                lb = lba if aux else lbm
                x0 = xp.tile([128, TP], F32, tag="x")
                nc.sync.dma_start(out=x0[:MEL, :], in_=ins["xm"][b])
                x = [x0]
                for l in range(NL):
                    cin_tiles = 1 if l == 0 else 2
                    kdim = MEL if l == 0 else 128
                    wts = []
                    for k in range(3):
                        for cit in range(cin_tiles):
                            w = wtp.tile([128, C], F32, tag="wt")
                            if l == 0:
                                nc.sync.dma_start(out=w[:MEL, :], in_=w0[k])
                            else:
                                nc.sync.dma_start(out=w, in_=wr[l - 1, k, cit * 128:(cit + 1) * 128, :])
                            wts.append(w)
                    xn = []
                    for ct in range(2):
                        stats = stp.tile([128, 2, 6], F32, tag="bnst")
                        pss = []
                        for lc in range(2):
                            ps = psp.tile([128, 512], F32, tag="cps")
                            mm = 0
                            for k in range(3):
                                for cit in range(cin_tiles):
                                    nc.tensor.matmul(
                                        ps,
                                        wts[k * cin_tiles + cit][:kdim, ct * 128:(ct + 1) * 128],
                                        x[cit][:kdim, lc * 512 + k: lc * 512 + k + 512],
                                        start=(mm == 0), stop=(mm == 3 * cin_tiles - 1))
                                    mm += 1
                            nc.vector.bn_stats(out=stats[:, lc, :], in_=ps[:, :])
                            pss.append(ps)
                        mv = stp.tile([128, 2], F32, tag="bnmv")
                        nc.vector.bn_aggr(out=mv, in_=stats)
                        sd = stp.tile([128, 1], F32, tag="sd")
                        nc.scalar.activation(out=sd, in_=mv[:, 1:2], func=AF.Sqrt, bias=epsc)
                        rs = stp.tile([128, 1], F32, tag="rs")
                        nc.vector.reciprocal(rs, sd)
                        ga = stp.tile([128, 1], F32, tag="ga")
                        nc.vector.tensor_mul(ga, rs, gw[:, l * 2 + ct: l * 2 + ct + 1])
                        gbb = stp.tile([128, 1], F32, tag="gbb")
                        nc.vector.tensor_mul(gbb, mv[:, 0:1], ga)
                        nc.vector.tensor_sub(gbb, gb[:, l * 2 + ct: l * 2 + ct + 1], gbb)
                        xt_ = xp.tile([128, TP], F32, tag="x")
                        for lc in range(2):
                            nc.scalar.activation(out=xt_[:, 1 + lc * 512: 1 + (lc + 1) * 512],
                                                 in_=pss[lc], func=AF.Relu, scale=ga, bias=gbb)
                        lcol = lb[:, (l * NB + b) * 2 + ct: (l * NB + b) * 2 + ct + 1]
                        if l % 2 == 1:
                            # (lbias + y) + x_prev, then mask
                            nc.vector.scalar_tensor_tensor(
                                out=xt_[:, 1:1 + TDEC], in0=xt_[:, 1:1 + TDEC], scalar=lcol,
                                in1=x[ct][:, 1:1 + TDEC], op0=OP.add, op1=OP.add)
                            if l < NL - 1:
                                nc.vector.tensor_mul(xt_[:, 1:1 + TDEC], xt_[:, 1:1 + TDEC],
                                                     maskbc[b])
                        else:
                            # (lbias + y) * mask in one op
                            if l < NL - 1:
                                nc.vector.scalar_tensor_tensor(
                                    out=xt_[:, 1:1 + TDEC], in0=xt_[:, 1:1 + TDEC], scalar=lcol,
                                    in1=maskbc[b], op0=OP.add, op1=OP.mult)
                            else:
                                nc.vector.tensor_scalar_add(xt_[:, 1:1 + TDEC],
                                                            xt_[:, 1:1 + TDEC], lcol)
                        nc.vector.memset(xt_[:, 0:1], 0.0)
                        nc.vector.memset(xt_[:, TP - 1: TP], 0.0)
                        xn.append(xt_)
                    x = xn
                return x

            def normalize_rows(t, n_free):
                scr = rot.tile([128, n_free], F32, tag="sw")
                sq = stp.tile([128, 1], F32, tag="nsq")
                nc.vector.tensor_mul(scr, t[:, :n_free], t[:, :n_free])
                nc.vector.reduce_sum(sq, scr, axis=AX.X)
                sd = stp.tile([128, 1], F32, tag="nsd")
                nc.scalar.activation(out=sd, in_=sq, func=AF.Sqrt)
                rr = stp.tile([128, 1], F32, tag="nrr")
                nc.vector.reciprocal(rr, sd)
                nc.vector.tensor_scalar_mul(t[:, :n_free], t[:, :n_free], rr)

            def stage_val(val, b, m, xt):
                p = 2 * m + b
                nc.sync.dma_start(out=d_val[p, xt * 128:(xt + 1) * 128, :], in_=val)

            # ---------------- main stacks, key/query, similarity ----------------
            m2_tiles = {}
            x6a_all = {}
            for b in range(NB):
                x6 = conv_stack(b, aux=False)
                if P >= 2:
                    x6a_all[b] = conv_stack(b, aux=True)
                if P < 1:
                    continue
                SUB = int(os.environ.get("KSUB", "9"))

                # key
                tta = [per.tile([128, TX], F32, tag=f"tt{i}", name=f"tt{i}") for i in range(2)]
                ttb = per.tile([1, TX], F32, tag="ttb")
                for i in range(2):
                    nc.sync.dma_start(out=tta[i], in_=ins["textT"][b, i * 128:(i + 1) * 128, :])
                nc.sync.dma_start(out=ttb, in_=ins["textT"][b, 256:257, :])
                if b == 0:
                    eW = [per.tile([128, ATT_H], F32, tag=f"eW{i}", name=f"eW{i}") for i in range(2)]
                    eWb = per.tile([1, ATT_H], F32, tag="eWb")
                    for i in range(2):
                        nc.sync.dma_start(out=eW[i], in_=ins["eWt"][i * 128:(i + 1) * 128, :])
                    nc.sync.dma_start(out=eWb, in_=ins["eWt"][256:257, :])
                    qW = [per.tile([128, ATT_H - 1], F32, tag=f"qW{i}", name=f"qW{i}") for i in range(2)]
                    qWb = per.tile([1, ATT_H - 1], F32, tag="qWb")
                    for i in range(2):
                        nc.sync.dma_start(out=qW[i], in_=ins["qWt"][i * 128:(i + 1) * 128, :])
                    nc.sync.dma_start(out=qWb, in_=ins["qWt"][256:257, :])
                keyT = per.tile([128, TX], F32, tag=f"keyT{b}")
                nc.vector.memset(keyT, 0.0)
                for xt in range(2):
                    if SUB < 1:
                        continue
                    pk = psp2.tile([128, ATT_H], F32, tag="sp")
                    for i in range(2):
                        nc.tensor.matmul(pk, tta[i][:, xt * 128:(xt + 1) * 128], eW[i],
                                         start=(i == 0), stop=False)
                    nc.tensor.matmul(pk, ttb[:, xt * 128:(xt + 1) * 128], eWb,
                                     start=False, stop=True)
                    kk = rot.tile([128, ATT_H], F32, tag="sw")
                    nc.scalar.activation(out=kk, in_=pk, func=AF.Copy)
                    if SUB >= 2:
                        normalize_rows(kk, ATT_H)
                    if SUB >= 3:
                        pt = psp2.tile([128, 128], F32, tag="sp")
                        nc.tensor.transpose(pt, kk, ident)
                        nc.scalar.activation(out=keyT[:, xt * 128:(xt + 1) * 128], in_=pt, func=AF.Copy)

                if SUB < 4:
                    continue
                # query
                qT = per.tile([128, TDEC], F32, tag="qT")
                for yt in range(8):
                    pq = psp2.tile([128, ATT_H - 1], F32, tag="sp")
                    for i in range(2):
                        nc.tensor.matmul(pq, x6[i][:, 1 + yt * 128: 1 + (yt + 1) * 128], qW[i],
                                         start=(i == 0), stop=False)
                    nc.tensor.matmul(pq, ones_row, qWb,
                                     start=False, stop=True)
                    qq = rot.tile([128, 128], F32, tag="sw")
                    nc.scalar.activation(out=qq[:, 0:ATT_H - 1], in_=pq, func=AF.Copy)
                    sy = rot.tile([128, MEL], F32, tag="sw")
                    nc.sync.dma_start(out=sy, in_=ins["spec_y"][b, yt])
                    en = stp.tile([128, 1], F32, tag="en")
                    nc.vector.reduce_sum(en, sy, axis=AX.X)
                    nc.vector.tensor_scalar_mul(qq[:, ATT_H - 1: ATT_H], en, 1.0 / MEL)
                    normalize_rows(qq, ATT_H)
                    pt = psp2.tile([128, 128], F32, tag="sp")
                    nc.tensor.transpose(pt, qq, ident)
                    nc.scalar.activation(out=qT[:, yt * 128:(yt + 1) * 128], in_=pt, func=AF.Copy)

                if SUB < 5:
                    continue
                # similarity + masked softplus sums + val1
                snps = []
                for xt in range(2):
                    bx = b * 2 + xt
                    ssim = simall[:, bx * TDEC:(bx + 1) * TDEC]
                    for yc in range(2):
                        psim = psp.tile([128, 512], F32, tag="cps")
                        nc.tensor.matmul(psim, keyT[:, xt * 128:(xt + 1) * 128],
                                         qT[:, yc * 512:(yc + 1) * 512], start=True, stop=True)
                        nc.scalar.activation(out=ssim[:, yc * 512:(yc + 1) * 512], in_=psim,
                                             func=AF.Identity, scale=sc10, bias=smb)
                    m2 = m2all[:, bx * TDEC:(bx + 1) * TDEC]
                    nc.vector.tensor_scalar_mul(m2, maskbc[b], tmcol[:, b * 2 + xt: b * 2 + xt + 1])
                    m2_tiles[(b, xt)] = m2
                    # nlsn = softplus(sim) = relu(sim) + ln(1 + exp(-|sim|))
                    ab = rot.tile([128, TDEC], F32, tag="work")
                    nc.scalar.activation(out=ab, in_=ssim, func=AF.Abs)
                    nc.scalar.activation(out=ab, in_=ab, func=AF.Exp, scale=-1.0)
                    nc.scalar.activation(out=ab, in_=ab, func=AF.Ln, bias=ones_col)
                    nlsn = rot.tile([128, TDEC], F32, tag="work")
                    nc.scalar.activation(out=nlsn, in_=ssim, func=AF.Relu)
                    nc.vector.tensor_add(nlsn, nlsn, ab)
                    scr = rot.tile([128, TDEC], F32, tag="work")
                    snp = stp.tile([128, 1], F32, tag=f"snp{xt}")
                    nc.vector.tensor_mul(scr, nlsn, m2)
                    nc.vector.reduce_sum(snp, scr, axis=AX.X)
                    snps.append(snp)
                    # softplus(-sim) = softplus(sim) - sim
                    lsn = rot.tile([128, TDEC], F32, tag="work")
                    nc.vector.tensor_sub(lsn, nlsn, ssim)
                    val = rot.tile([128, TDEC], F32, tag="work")
                    nc.vector.tensor_mul(val, lsn, m2)
                    stage_val(val, b, 0, xt)
                sn2 = stp.tile([128, 1], F32, tag="sn2")
                nc.vector.tensor_add(sn2, snps[0], snps[1])
                psn = psp2.tile([1, 1], F32, tag="sp")
                nc.tensor.matmul(psn, sn2, ones_col, start=True, stop=True)
                nc.scalar.activation(out=snall[:, b:b + 1], in_=psn, func=AF.Copy)

            # ---------------- aux stacks, ctc, sim_ctc, val2 ----------------
            for b in range(NB if P >= 2 else 0):
                x6a = x6a_all[b]
                if b == 0:
                    cW = [per.tile([128, VOCAB], F32, tag=f"cW{i}", name=f"cW{i}") for i in range(2)]
                    cWb = per.tile([1, VOCAB], F32, tag="cWb")
                    for i in range(2):
                        nc.sync.dma_start(out=cW[i], in_=ins["cWt"][i * 128:(i + 1) * 128, :])
                    nc.sync.dma_start(out=cWb, in_=ins["cWt"][256:257, :])
                qTc = per.tile([128, TDEC], F32, tag="qTc")
                for yt in range(8):
                    pc = psp2.tile([128, VOCAB], F32, tag="sp")
                    for i in range(2):
                        nc.tensor.matmul(pc, x6a[i][:, 1 + yt * 128: 1 + (yt + 1) * 128], cW[i],
                                         start=(i == 0), stop=False)
                    nc.tensor.matmul(pc, ones_row, cWb,
                                     start=False, stop=True)
                    mx = stp.tile([128, 1], F32, tag="mx")
                    nc.vector.reduce_max(mx, pc, axis=AX.X)
                    nmx = stp.tile([128, 1], F32, tag="nmx")
                    nc.vector.tensor_scalar_mul(nmx, mx, -1.0)
                    exb = rot.tile([128, VOCAB], F32, tag="sw")
                    nc.scalar.activation(out=exb, in_=pc, func=AF.Exp, bias=nmx)
                    s = stp.tile([128, 1], F32, tag="s")
                    nc.vector.reduce_sum(s, exb, axis=AX.X)
                    lns = stp.tile([128, 1], F32, tag="lns")
                    nc.scalar.activation(out=lns, in_=s, func=AF.Ln)
                    r = stp.tile([128, 1], F32, tag="r")
                    nc.vector.reciprocal(r, s)
                    qsb = rot.tile([128, VOCAB], F32, tag="sw")
                    nc.vector.tensor_scalar_mul(qsb, exb, r)
                    nb_ = stp.tile([128, 1], F32, tag="nb_")
                    nc.vector.tensor_add(nb_, mx, lns)
                    nc.vector.tensor_scalar_mul(nb_, nb_, -1.0)
                    lidx = b * 8 + yt
                    nc.scalar.activation(out=lsmall[:, lidx * VOCAB:(lidx + 1) * VOCAB],
                                         in_=pc, func=AF.Identity, bias=nb_)
                    ptc = psp2.tile([128, 128], F32, tag="sp")
                    nc.tensor.transpose(ptc[:VOCAB, :], qsb, ident)
                    nc.scalar.activation(out=qTc[:VOCAB, yt * 128:(yt + 1) * 128],
                                         in_=ptc[:VOCAB, :], func=AF.Copy)
                eh = per.tile([128, TX], F32, tag="eh")
                nc.sync.dma_start(out=eh[:VOCAB, :], in_=ins["ehotT"][b])
                for xt in range(2):
                    scs = rot.tile([128, TDEC], F32, tag="work")
                    for yc in range(2):
                        psc = psp.tile([128, 512], F32, tag="cps")
                        nc.tensor.matmul(psc, eh[:VOCAB, xt * 128:(xt + 1) * 128],
                                         qTc[:VOCAB, yc * 512:(yc + 1) * 512],
                                         start=True, stop=True)
                        nc.scalar.activation(out=scs[:, yc * 512:(yc + 1) * 512], in_=psc,
                                             func=AF.Copy)
                    # softplus(-simc) = relu(-simc) + ln(1 + exp(-|simc|))
                    ab2 = rot.tile([128, TDEC], F32, tag="work")
                    nc.scalar.activation(out=ab2, in_=scs, func=AF.Abs)
                    nc.scalar.activation(out=ab2, in_=ab2, func=AF.Exp, scale=-1.0)
                    nc.scalar.activation(out=ab2, in_=ab2, func=AF.Ln, bias=ones_col)
                    lsn2 = rot.tile([128, TDEC], F32, tag="work")
                    nc.scalar.activation(out=lsn2, in_=scs, func=AF.Relu, scale=-1.0)
                    nc.vector.tensor_add(lsn2, lsn2, ab2)
                    val2 = rot.tile([128, TDEC], F32, tag="work")
                    nc.vector.tensor_mul(val2, lsn2, m2_tiles[(b, xt)])
                    stage_val(val2, b, 1, xt)

            # ---------------- MAS forward scans ----------------

            # rows beyond 223 can never be touched: text_lengths < 225 and the DP
            # flows strictly downward in i, so cap the scan row count.
            TX_SCAN = 224
            for i in range(TX_SCAN if P >= 3 else 0):
                s = i % NSLOT
                sv = i % VRING
                nc.sync.dma_start(out=valring[0:4, sv * TDEC:(sv + 1) * TDEC],
                                  in_=d_val[0:4, i, :])
                if i == 0:
                    data0 = slotring[0:4, (NSLOT - 1) * VB: (NSLOT - 1) * VB + TDEC]
                    init = 0.0
                else:
                    sp_ = (i - 1) % NSLOT
                    data0 = slotring[0:4, sp_ * VB: sp_ * VB + TDEC]
                    init = BIGPOS
                nc.vector.tensor_tensor_scan(
                    out=slotring[0:4, s * VB + 1: s * VB + 1 + TDEC],
                    data0=data0,
                    data1=valring[0:4, sv * TDEC:(sv + 1) * TDEC],
                    initial=init, op0=OP.min, op1=OP.add)
                nc.sync.dma_start(out=dvt[i // 128][i % 2][0:4, (i % 128) // 2, :],
                                  in_=slotring[0:4, s * VB: (s + 1) * VB])

            # ---------------- D bits ----------------
            dbu = per.tile([128, 8 * TDEC], U8, tag="dbu")
            for g in range(2 if P >= 4 else 0):
                band = rot.tile([128, TDEC], F32, tag="work", name=f"band{g}")
                nc.vector.tensor_scalar(band, jio, rix[:, g:g + 1], None, op0=OP.is_equal)
                for p in range(4):
                    Ab = rot.tile([128, VB], F32, tag="ab", name=f"Ab{g}{p}")
                    Bb = rot.tile([128, VB], F32, tag="ab", name=f"Bb{g}{p}")
                    nr = 64 if g == 0 else 48  # rows per parity in this block
                    if g == 1:
                        nc.vector.memset(Ab[96:128, :], 0.0)
                        nc.vector.memset(Bb[96:128, :], 0.0)
                    nc.sync.dma_start(out=Ab[0:2 * nr:2, :], in_=dvt[g][0][p, 0:nr, :])
                    nc.sync.dma_start(out=Ab[1:2 * nr:2, :], in_=dvt[g][1][p, 0:nr, :])
                    if g == 0:
                        nc.vector.memset(Bb[0:1, :], 0.0)
                    else:
                        nc.sync.dma_start(out=Bb[0:1, :], in_=dvt[0][1][p, 63:64, :])
                    nc.sync.dma_start(out=Bb[1:2 * nr:2, :], in_=dvt[g][0][p, 0:nr, :])
                    nc.sync.dma_start(out=Bb[2:2 * nr:2, :], in_=dvt[g][1][p, 0:nr - 1, :])
                    cmpf = rot.tile([128, TDEC], F32, tag="work", name=f"cmpf{g}{p}")
                    nc.vector.tensor_tensor(out=cmpf, in0=Bb[:, 0:TDEC], in1=Ab[:, 0:TDEC],
                                            op=OP.is_lt)
                    nc.vector.tensor_tensor(out=dbu[:, (g * 4 + p) * TDEC:(g * 4 + p + 1) * TDEC],
                                            in0=cmpf, in1=band, op=OP.max)
            nc.sync.dma_start(out=outs["o_db"][:, :], in_=dbu)
            # out APs iterate (q, c, inner) to match the sbuf staging layout
            sim_ap = bass.AP(outs["o_sim"], 0, [[TDEC, 128], [2 * 128 * TDEC, NB], [128 * TDEC, 2], [1, TDEC]])
            nc.sync.dma_start(out=sim_ap, in_=simall.rearrange("p (c t) -> p c t", c=4))
            mask_ap = bass.AP(outs["o_mask"], 0, [[TDEC, 128], [2 * 128 * TDEC, NB], [128 * TDEC, 2], [1, TDEC]])
            nc.sync.dma_start(out=mask_ap, in_=m2all.rearrange("p (c t) -> p c t", c=4))
            lsm_ap = bass.AP(outs["o_lsm"], 0, [[VOCAB, 128], [8 * 128 * VOCAB, NB], [128 * VOCAB, 8], [1, VOCAB]])
            nc.sync.dma_start(out=lsm_ap, in_=lsmall.rearrange("p (c v) -> p c v", c=16))
            nc.sync.dma_start(out=outs["o_snls"][:, :], in_=snall)

    nc.finalize()
    return nc


# ----------------------------------------------------------------------------
# host: input prep
# ----------------------------------------------------------------------------

def _relu(x):
    return np.maximum(x, 0.0)


def _mlp2(v, w1, b1, w2, b2):
    h = _relu(v @ w1.T + b1)
    return _relu(h @ w2.T + b2)


def _prep_shared(params):
    p = {}
    aug = lambda w, b: np.ascontiguousarray(
        np.concatenate([np.asarray(w, np.float32).T, np.asarray(b, np.float32)[None, :]], 0))
    p["eWt"] = aug(params["enc_proj_w"], params["enc_proj_b"])
    p["qWt"] = aug(params["query_proj_w"], params["query_proj_b"])
    p["cWt"] = aug(params["ctc_proj_w"], params["ctc_proj_b"])
    for stk, tag in ((params["main"], "m"), (params["aux"], "a")):
        p[f"w0t_{tag}"] = np.ascontiguousarray(
            np.asarray(stk[0]["conv_w"], np.float32).transpose(2, 1, 0))
        p[f"wrt_{tag}"] = np.ascontiguousarray(np.stack(
            [np.asarray(stk[l]["conv_w"], np.float32).transpose(2, 1, 0) for l in range(1, NL)]))
        p[f"gnw_{tag}"] = np.ascontiguousarray(
            np.stack([np.asarray(stk[l]["gn_w"], np.float32) for l in range(NL)]))
        p[f"gnb_{tag}"] = np.ascontiguousarray(
            np.stack([np.asarray(stk[l]["gn_b"], np.float32) for l in range(NL)]))
    p["consts"] = np.array(
        [[10.0 * np.exp(np.float32(params["sim_w"])), np.float32(params["sim_b"]),
          1e-5, 0, 0, 0, 0, 0]], np.float32)
    ridx = np.empty((128, 8), np.float32)
    for pp in range(128):
        for g in range(8):
            ridx[pp, g] = 128 * g + pp
    p["rowidx"] = ridx
    return p


def _prep_core(c, text, spec, spkr_vec, gst_vec, text_lengths, spec_lengths,
               enc_input, params, shared):
    sl = slice(2 * c, 2 * c + 2)
    spec_c = np.asarray(spec[sl], np.float32)
    tl = np.asarray(text_lengths[sl]).astype(np.int64)
    sll = np.asarray(spec_lengths[sl]).astype(np.int64)
    smask = (np.arange(TDEC)[None, :] < sll[:, None]).astype(np.float32)
    d = dict(shared)
    xm = np.zeros((NB, MEL, TP), np.float32)
    xm[:, :, 1:1 + TDEC] = (spec_c * smask[:, :, None]).transpose(0, 2, 1)
    d["xm"] = xm
    d["spec_y"] = np.ascontiguousarray(spec_c.reshape(NB, 8, 128, MEL))
    tt = np.empty((NB, 257, TX), np.float32)
    tt[:, :256] = np.asarray(text[sl], np.float32).transpose(0, 2, 1)
    tt[:, 256] = 1.0
    d["textT"] = tt
    sm = np.zeros((NB, TP), np.float32)
    sm[:, 1:1 + TDEC] = smask
    d["smask"] = sm
    d["tmask"] = (np.arange(TX)[None, :] < tl[:, None]).astype(np.float32)
    eh = np.zeros((NB, VOCAB, TX), np.float32)
    ei = np.asarray(enc_input[sl]).astype(np.int64)
    for b in range(NB):
        eh[b, ei[b], np.arange(TX)] = 1.0
    d["ehotT"] = eh
    sv = np.asarray(spkr_vec[sl], np.float32)
    gv = np.asarray(gst_vec[sl], np.float32)
    for stk, tag in ((params["main"], "m"), (params["aux"], "a")):
        lb = np.empty((NL, NB, C), np.float32)
        for l in range(NL):
            pl = stk[l]
            lb[l] = (_mlp2(sv, np.asarray(pl["s1w"], np.float32), np.asarray(pl["s1b"], np.float32),
                           np.asarray(pl["s2w"], np.float32), np.asarray(pl["s2b"], np.float32))
                     + _mlp2(gv, np.asarray(pl["g1w"], np.float32), np.asarray(pl["g1b"], np.float32),
                             np.asarray(pl["g2w"], np.float32), np.asarray(pl["g2b"], np.float32)))
        d[f"lb_{tag}"] = lb
    return d


# ----------------------------------------------------------------------------
# host: post-processing
# ----------------------------------------------------------------------------

def _backtrack(D, t_x, t_y):
    """D [B,TX,TDEC] uint8; returns idx trajectories [B, TDEC] int64."""
    B = D.shape[0]
    bi = np.arange(B)
    index = (t_x - 1).astype(np.int64).copy()
    idx_traj = np.empty((B, TDEC), np.int64)
    for j in range(TDEC - 1, -1, -1):
        idx_traj[:, j] = index
        write = j < t_y
        move = (index != 0) & (D[bi, index, j] != 0)
        index = np.where(write & move, index - 1, index)
    return idx_traj


def _ctc_loss(log_probs_nt, targets, input_lengths, target_lengths, blank=0):
    """log_probs_nt [N, T, V] f32. Reference-faithful log-space CTC."""
    Nb, T, V = log_probs_nt.shape
    S = targets.shape[1]
    L = 2 * S + 1
    NEG = -1e9
    ext = np.full((Nb, L), blank, np.int64)
    ext[:, 1::2] = targets
    skip = np.concatenate([np.zeros((Nb, 2), bool),
                           (ext[:, 2:] != blank) & (ext[:, 2:] != ext[:, :-2])], axis=1)
    lp_ext = np.take_along_axis(log_probs_nt, np.broadcast_to(ext[:, None, :], (Nb, T, L)), axis=2)
    lp_ext = np.ascontiguousarray(lp_ext.transpose(1, 0, 2), dtype=np.float32)  # [T,N,L]
    alpha = np.full((Nb, L), NEG, np.float32)
    alpha[:, 0] = lp_ext[0, :, 0]
    alpha[:, 1] = lp_ext[0, :, 1]
    a2 = np.empty_like(alpha)
    a3 = np.empty_like(alpha)
    for t in range(1, T):
        a2[:, 0] = NEG
        a2[:, 1:] = alpha[:, :-1]
        a3[:, :2] = NEG
        a3[:, 2:] = np.where(skip[:, 2:], alpha[:, :-2], NEG)
        new = (np.logaddexp(np.logaddexp(alpha, a2), a3) + lp_ext[t]).astype(np.float32)
        upd = t < input_lengths
        alpha[upd] = new[upd]
    bi = np.arange(Nb)
    e1 = alpha[bi, 2 * target_lengths]
    e2 = alpha[bi, 2 * target_lengths - 1]
    loss = -np.logaddexp(e1, e2)
    return np.float32(np.mean(loss / target_lengths.astype(np.float32)))


_NC_CACHE = None


def kernel(text, spec, text_lengths, spec_lengths, spkr_vec, gst_vec, enc_input, params):
    global _NC_CACHE
    text = np.asarray(text)
    spec = np.asarray(spec)
    text_lengths = np.asarray(text_lengths)
    spec_lengths = np.asarray(spec_lengths)
    enc_input = np.asarray(enc_input)

    if _NC_CACHE is None:
        _NC_CACHE = build_nc()
    nc = _NC_CACHE

    shared = _prep_shared(params)
    in_maps = [
        _prep_core(c, text, spec, spkr_vec, gst_vec, text_lengths, spec_lengths,
                   enc_input, params, shared)
        for c in range(NCORES)
    ]
    res = run_bass_kernel_spmd(nc, in_maps, core_ids=list(range(NCORES)))
    results = res.results

    tl = text_lengths.astype(np.int64)
    sl = spec_lengths.astype(np.int64)
    sim = np.empty((N, TX, TDEC), np.float32)
    att_mask = np.empty((N, TX, TDEC), np.float32)
    lsm = np.empty((N, TDEC, VOCAB), np.float32)
    snls = np.empty((N,), np.float32)
    D1 = np.empty((N, TX, TDEC), np.uint8)
    D2 = np.empty((N, TX, TDEC), np.uint8)
    for c in range(NCORES):
        r = results[c]
        s2 = slice(2 * c, 2 * c + 2)
        sim[s2] = r["o_sim"].reshape(NB, TX, TDEC)
        att_mask[s2] = r["o_mask"].reshape(NB, TX, TDEC)
        lsm[s2] = r["o_lsm"].reshape(NB, TDEC, VOCAB)
        snls[s2] = r["o_snls"][:, 0]
        Db = r["o_db"].reshape(128, 2, 4, TDEC).transpose(2, 1, 0, 3).reshape(4, TX, TDEC)
        D1[2 * c] = Db[0]
        D1[2 * c + 1] = Db[1]
        D2[2 * c] = Db[2]
        D2[2 * c + 1] = Db[3]

    idx1 = _backtrack(D1, tl, sl)
    idx2 = _backtrack(D2, tl, sl)

    bi = np.arange(N)[:, None]
    jj = np.arange(TDEC)[None, :]
    wmask = jj < sl[:, None]

    attention = np.zeros((N, TX, TDEC), np.float32)
    bidx, jidx = np.nonzero(wmask)
    attention[bidx, idx1[bidx, jidx], jidx] = 1.0

    # losses
    denom = (tl * sl).astype(np.float32)
    path_sim = np.sum(sim[bi, idx1, jj] * wmask, axis=1, dtype=np.float64).astype(np.float32)
    icl = (snls - path_sim) / denom
    nll = np.float32(np.mean(icl))

    aux_sim = np.zeros((N,), np.float64)
    for dshift in (-1, 0, 1):
        ii = idx2 + dshift
        ok = wmask & (ii >= 0) & (ii < tl[:, None])
        aux_sim += np.sum(sim[bi, np.clip(ii, 0, TX - 1), jj] * ok, axis=1, dtype=np.float64)
    aux_l = (snls - aux_sim.astype(np.float32)) / denom * 0.5

    ctc = _ctc_loss(lsm, enc_input.astype(np.int64), sl, tl)
    att_loss = np.float32(nll + np.float32(np.mean(aux_l)) + ctc)

    return attention, att_loss, att_mask, np.float32(nll)
